# revision 1
# baseline (speedup 1.0000x reference)
# Trainium2 Bass kernel for nn_KokoroModel (text->mel seq2seq, LSTM enc/dec + MHA).
#
# Sharding: data-parallel over batch, 4 examples/core on 8 cores, weights
# replicated. Host-side weight-only fusions (model reparametrizations):
#   TAB  = emb @ enc_Wih.T            (embedding gather -> one-hot matmul)
#   QW   = (Wq @ mproj_in_W) / 8      (mel proj + Q proj + attn scale)
#   M1   = dec_Wih[:, :H] @ mproj_in_W
#   M2   = dec_Wih[:, H:] @ attn_out_W
#   K/V biases absorb tproj_b; decoder gate bias absorbs mproj_in/attn_out biases.
# Softmax normalization is deferred: exp-scores stay unnormalized, row-sums come
# from a ones-column appended to V, context is scaled by 1/sum afterwards
# (valid: scores are tiny, exp cannot overflow).
#
# LSTM steps run weights-stationary on PE: 64 (ldweights+matmul) pairs/step
# accumulate h @ Whh.T into PSUM[128, 16*n] with gates partition-major, then
# ACT sigmoid/tanh and DVE gate algebra. Gate order is torch's [i,f,g,o] in
# 128-row blocks, so sigmoid covers two contiguous spans and tanh one.

import numpy as np
import ml_dtypes

import concourse.bass as bass
import concourse.tile as tile
from concourse import bacc, mybir
from concourse import bass_utils

F32 = mybir.dt.float32
BF16 = mybir.dt.bfloat16
AF = mybir.ActivationFunctionType
BF = ml_dtypes.bfloat16

VOCAB, MEL, H = 256, 80, 512
NH, HD = 8, 64
B_FULL, S_FULL, T_FULL = 32, 512, 1000
NCORES = 8


# ---------------------------------------------------------------------------
# host-side layout helpers
# ---------------------------------------------------------------------------

def _lhsT_tiles(w, kp=128, mp=128):
    """w: [K, M] -> [kp, (K//kp)*(M//mp)*mp]; tile (kc,mc) at cols
    (kc*(M//mp)+mc)*mp."""
    K, M = w.shape
    nk, nm = K // kp, M // mp
    return np.ascontiguousarray(
        w.reshape(nk, kp, nm, mp).transpose(1, 0, 2, 3).reshape(kp, nk * nm * mp))


def _bias_tiles(b, p=128):
    G = b.shape[0]
    return np.ascontiguousarray(b.reshape(G // p, p).T.astype(np.float32))


# ---------------------------------------------------------------------------
# device program
# ---------------------------------------------------------------------------

def build_program(S=S_FULL, T=T_FULL, n=B_FULL // NCORES, stages=5):
    nc = bacc.Bacc("TRN2", target_bir_lowering=False, debug=False)

    NS, NT = n * S, n * T
    SN = S            # text matmul N-tile (== S so each tile is one example)
    TN = T // 2       # mel matmul N-tile (1000 -> 500)
    assert SN <= 512 and TN <= 512 and TN * 4 <= 2048

    d = {}

    def din(name, shape, dt):
        d[name] = nc.dram_tensor(name, list(shape), dt, kind="ExternalInput")

    din("oh_f", (128, 2 * NS), BF16)
    din("oh_b", (128, 2 * NS), BF16)
    din("tab_f", (128, 2 * 2048), BF16)
    din("tab_b", (128, 2 * 2048), BF16)
    din("whh_f", (128, 4 * 16 * 128), BF16)
    din("whh_b", (128, 4 * 16 * 128), BF16)
    din("whh_d", (128, 4 * 16 * 128), BF16)
    din("wt_f", (128, 4 * 4 * 128), BF16)
    din("wt_b", (128, 4 * 4 * 128), BF16)
    din("tb", (128, 4), F32)
    din("wk", (128, 4 * 4 * 128), BF16)
    din("wv", (128, 4 * 512), BF16)
    din("vb", (64, 8), F32)
    din("qw", (81, 4 * 128), BF16)
    din("melT", (81, NT), BF16)
    din("m1", (81, 2048), BF16)
    din("m2", (128, 4 * 2048), BF16)
    din("mo", (128, 4 * 80), BF16)
    din("mob", (80, 1), F32)

    out = nc.dram_tensor("out", [80, NT], F32, kind="ExternalOutput")

    xg_f = nc.dram_tensor("xg_f", [NS, 2048], BF16, kind="Internal")
    xg_b = nc.dram_tensor("xg_b", [NS, 2048], BF16, kind="Internal")
    xg_d = nc.dram_tensor("xg_d", [NT, 2048], BF16, kind="Internal")

    with tile.TileContext(nc) as tc:
        _body(tc, nc, d, out, xg_f, xg_b, xg_d, S, T, n, NS, NT, SN, TN, stages)

    nc.compile()
    return nc


def _bias_copy(nc, i, dst, src, bias_col):
    """psum->sbuf cast + per-partition bias; alternate ACT/DVE for throughput."""
    if i % 2 == 0:
        nc.scalar.activation(dst, src, AF.Identity, bias=bias_col)
    else:
        nc.vector.tensor_scalar_add(dst, src, bias_col)


def _plain_copy(nc, i, dst, src):
    if i % 2 == 0:
        nc.scalar.copy(dst, src)
    else:
        nc.vector.tensor_copy(dst, src)


def _load(nc, pool, d, name, shape, dt):
    t_ = pool.tile(list(shape), dt, tag=name)
    nc.sync.dma_start(t_[:, :], d[name].ap())
    return t_


def _xg_produce(tc, nc, psp, sbp, tab, oh, xg_dram, S, n, NS, SN):
    """xg[b*S+t, g] = (onehot.T @ TAB')[token, g]; bias pre-folded into TAB'.
    Token-major layout: contiguous DMA runs both directions."""
    i = 0
    for b in range(n):
        for tcx in range(S // 128):
            row0 = b * S + tcx * 128
            for gs in range(4):
                ps = psp.tile([128, 512], F32, tag="ps")
                for vc in range(2):
                    nc.tensor.matmul(
                        ps[:, :],
                        oh[:, vc * NS + row0:vc * NS + row0 + 128],
                        tab[:, vc * 2048 + gs * 512:vc * 2048 + (gs + 1) * 512],
                        start=(vc == 0), stop=(vc == 1))
                sb = sbp.tile([128, 512], BF16, tag="xg_sb")
                _plain_copy(nc, i, sb[:, :], ps[:, :])
                i += 1
                nc.sync.dma_start(
                    xg_dram.ap()[row0:row0 + 128, gs * 512:(gs + 1) * 512], sb[:, :])


def _lstm(pools, nc, T_steps, n, xg_dram, whh, outbuf, T_buf, col_of_t, PF=8):
    """LSTM recurrence; h_t (bf16) lands in outbuf[:, kc*(T_buf*n) + b*T_buf +
    col_of_t(t)].  xg_dram: [T,16,128,n], gate blocks [i,f,g,o] x 4 chunks."""
    psum_p, state_p, gate_p, h_p, tmp_p, xg_p = pools
    xg_ap = xg_dram.ap().rearrange("(b t) (gc p) -> p t gc b", b=n, gc=16)
    ob = outbuf[:, :].rearrange("p (kc b t) -> p kc b t", kc=4, b=n)

    c = state_p.tile([128, 4 * n], F32, tag="c_state")
    nc.vector.memset(c[:, :], 0.0)
    h = state_p.tile([128, 4 * n], BF16, tag="h_init")
    nc.vector.memset(h[:, :], 0.0)

    gw = 4 * n
    xgs = None
    for t in range(T_steps):
        if t % PF == 0:
            pf = min(PF, T_steps - t)
            xgs = xg_p.tile([128, PF * 16 * n], BF16, tag="xg_stream")
            xv = xgs[:, :].rearrange("p (t g b) -> p t g b", t=PF, g=16)
            for b in range(n):
                nc.sync.dma_start(xv[:, :pf, :, b], xg_ap[:, t:t + pf, :, b])
        toff = (t % PF) * 16 * n

        ps = psum_p.tile([128, 16 * n], F32, tag="ps")
        for gc in range(16):
            for kc in range(4):
                nc.tensor.matmul(
                    ps[:, gc * n:(gc + 1) * n],
                    whh[:, (kc * 16 + gc) * 128:(kc * 16 + gc + 1) * 128],
                    h[:, kc * n:(kc + 1) * n],
                    start=(kc == 0), stop=(kc == 3))

        gates = gate_p.tile([128, 16 * n], F32, tag="gates")
        nc.vector.tensor_add(gates[:, :], ps[:, :], xgs[:, toff:toff + 16 * n])
        acts = gate_p.tile([128, 16 * n], F32, tag="acts")
        nc.scalar.activation(acts[:, 0:2 * gw], gates[:, 0:2 * gw], AF.Sigmoid)
        nc.scalar.activation(acts[:, 2 * gw:3 * gw], gates[:, 2 * gw:3 * gw], AF.Tanh)
        nc.scalar.activation(acts[:, 3 * gw:4 * gw], gates[:, 3 * gw:4 * gw], AF.Sigmoid)

        t1 = tmp_p.tile([128, gw], F32, tag="t1")
        nc.vector.tensor_mul(t1[:, :], acts[:, gw:2 * gw], c[:, :])
        t2 = tmp_p.tile([128, gw], F32, tag="t2")
        nc.vector.tensor_mul(t2[:, :], acts[:, 0:gw], acts[:, 2 * gw:3 * gw])
        nc.vector.tensor_add(c[:, :], t1[:, :], t2[:, :])
        tnh = tmp_p.tile([128, gw], F32, tag="tanh_c")
        nc.scalar.activation(tnh[:, :], c[:, :], AF.Tanh)
        h_new = h_p.tile([128, gw], BF16, tag="h_new")
        nc.vector.tensor_mul(h_new[:, :], acts[:, 3 * gw:4 * gw], tnh[:, :])

        nc.sync.dma_start(ob[:, :, :, col_of_t(t)],
                          h_new[:, :].rearrange("p (kc b) -> p kc b", kc=4))
        h = h_new


def _body(tc, nc, d, out, xg_f, xg_b, xg_d, S, T, n, NS, NT, SN, TN, stages=5):
    n_sn = NS // SN
    n_tn = NT // TN

    with tc.tile_pool(name="persist", bufs=1) as pp:
        melT = _load(nc, pp, d, "melT", (81, NT), BF16)
        te = pp.tile([128, 4 * NS], BF16, tag="te")
        ctx = pp.tile([128, 4 * NT], BF16, tag="ctx")

        # ---------------- phase E0: encoder gate precompute ----------------
        with tc.tile_pool(name="e0w", bufs=1) as wp, \
             tc.tile_pool(name="e0ps", bufs=4, space="PSUM") as psp, \
             tc.tile_pool(name="e0sb", bufs=4) as sbp:
            tab_f = _load(nc, wp, d, "tab_f", (128, 4096), BF16)
            tab_b = _load(nc, wp, d, "tab_b", (128, 4096), BF16)
            oh_f = _load(nc, wp, d, "oh_f", (128, 2 * NS), BF16)
            oh_b = _load(nc, wp, d, "oh_b", (128, 2 * NS), BF16)
            _xg_produce(tc, nc, psp, sbp, tab_f, oh_f, xg_f, S, n, NS, SN)
            _xg_produce(tc, nc, psp, sbp, tab_b, oh_b, xg_b, S, n, NS, SN)
        if stages <= 1:
            return

        # ---------------- phase E1+E2: encoder recurrences + tproj ---------
        with tc.tile_pool(name="e1w", bufs=1) as ewp, \
             tc.tile_pool(name="e1buf", bufs=1) as ebp, \
             tc.tile_pool(name="e1ps", bufs=4, space="PSUM") as pls, \
             tc.tile_pool(name="e1st", bufs=1) as stp, \
             tc.tile_pool(name="e1gt", bufs=3) as gtp, \
             tc.tile_pool(name="e1h", bufs=3) as hp, \
             tc.tile_pool(name="e1tmp", bufs=3) as tmp, \
             tc.tile_pool(name="e1xg", bufs=3) as xgp:

            whh_f = _load(nc, ewp, d, "whh_f", (128, 8192), BF16)
            whh_b = _load(nc, ewp, d, "whh_b", (128, 8192), BF16)
            buf_f = ebp.tile([128, 4 * NS], BF16, tag="buf_f")
            buf_b = ebp.tile([128, 4 * NS], BF16, tag="buf_b")

            lp = (pls, stp, gtp, hp, tmp, xgp)
            _lstm(lp, nc, S, n, xg_f, whh_f, buf_f, S, lambda t: t)
            _lstm(lp, nc, S, n, xg_b, whh_b, buf_b, S, lambda t: S - 1 - t)

            with tc.tile_pool(name="e2w", bufs=1) as wtp, \
                 tc.tile_pool(name="e2ps", bufs=4, space="PSUM") as ptp:
                wt_f = _load(nc, wtp, d, "wt_f", (128, 2048), BF16)
                wt_b = _load(nc, wtp, d, "wt_b", (128, 2048), BF16)
                tb = _load(nc, wtp, d, "tb", (128, 4), F32)
                for mc in range(4):
                    for nt in range(n_sn):
                        ps = ptp.tile([128, SN], F32, tag="ps")
                        for kc in range(4):
                            nc.tensor.matmul(
                                ps[:, :],
                                wt_f[:, (kc * 4 + mc) * 128:(kc * 4 + mc + 1) * 128],
                                buf_f[:, kc * NS + nt * SN:kc * NS + (nt + 1) * SN],
                                start=(kc == 0), stop=False)
                        for kc in range(4):
                            nc.tensor.matmul(
                                ps[:, :],
                                wt_b[:, (kc * 4 + mc) * 128:(kc * 4 + mc + 1) * 128],
                                buf_b[:, kc * NS + nt * SN:kc * NS + (nt + 1) * SN],
                                start=False, stop=(kc == 3))
                        _bias_copy(nc, mc * n_sn + nt,
                                   te[:, mc * NS + nt * SN:mc * NS + (nt + 1) * SN],
                                   ps[:, :], tb[:, mc:mc + 1])

        if stages <= 2:
            return
        # ---------------- phase A: attention ------------------------------
        _attention(tc, nc, d, te, ctx, melT, S, T, n, NS, NT, SN, TN)

        if stages <= 3:
            return
        # ---------------- phase X: decoder gate precompute ----------------
        with tc.tile_pool(name="xw", bufs=1) as dwp, \
             tc.tile_pool(name="xps", bufs=4, space="PSUM") as pdx, \
             tc.tile_pool(name="xsb", bufs=4) as dsb:
            m1 = _load(nc, dwp, d, "m1", (81, 2048), BF16)
            m2 = _load(nc, dwp, d, "m2", (128, 4 * 2048), BF16)
            tchunks = [(t0, min(128, T - t0)) for t0 in range(0, T, 128)]
            i = 0
            for b in range(n):
                for (t0, tsz) in tchunks:
                    row0 = b * T + t0
                    for gs in range(4):
                        ps = pdx.tile([128, 512], F32, tag="ps")
                        nc.tensor.matmul(
                            ps[:tsz, :], melT[:, row0:row0 + tsz],
                            m1[:, gs * 512:(gs + 1) * 512],
                            start=True, stop=False)
                        for kc in range(4):
                            nc.tensor.matmul(
                                ps[:tsz, :],
                                ctx[:, kc * NT + row0:kc * NT + row0 + tsz],
                                m2[:, kc * 2048 + gs * 512:kc * 2048 + (gs + 1) * 512],
                                start=False, stop=(kc == 3))
                        sb = dsb.tile([128, 512], BF16, tag="dx_sb")
                        _plain_copy(nc, i, sb[:tsz, :], ps[:tsz, :])
                        i += 1
                        nc.sync.dma_start(
                            xg_d.ap()[row0:row0 + tsz, gs * 512:(gs + 1) * 512],
                            sb[:tsz, :])

        if stages <= 4:
            return
        # ---------------- phase D: decoder recurrence + out proj ----------
        with tc.tile_pool(name="dw", bufs=1) as dwp, \
             tc.tile_pool(name="dbuf", bufs=1) as dbp, \
             tc.tile_pool(name="dps", bufs=4, space="PSUM") as pls, \
             tc.tile_pool(name="dst", bufs=1) as stp, \
             tc.tile_pool(name="dgt", bufs=3) as gtp, \
             tc.tile_pool(name="dh", bufs=3) as hp, \
             tc.tile_pool(name="dtmp", bufs=3) as tmp, \
             tc.tile_pool(name="dxg", bufs=3) as xgp:

            whh_d = _load(nc, dwp, d, "whh_d", (128, 8192), BF16)
            dbuf = dbp.tile([128, 4 * NT], BF16, tag="dbuf")
            lp = (pls, stp, gtp, hp, tmp, xgp)
            _lstm(lp, nc, T, n, xg_d, whh_d, dbuf, T, lambda t: t)

            with tc.tile_pool(name="ow", bufs=1) as mop, \
                 tc.tile_pool(name="ops", bufs=4, space="PSUM") as pso, \
                 tc.tile_pool(name="osb", bufs=4) as sbo:
                mo = _load(nc, mop, d, "mo", (128, 320), BF16)
                mob = _load(nc, mop, d, "mob", (80, 1), F32)
                for nt in range(n_tn):
                    ps = pso.tile([80, TN], F32, tag="ps")
                    for kc in range(4):
                        nc.tensor.matmul(
                            ps[:, :], mo[:, kc * 80:(kc + 1) * 80],
                            dbuf[:, kc * NT + nt * TN:kc * NT + (nt + 1) * TN],
                            start=(kc == 0), stop=(kc == 3))
                    sb = sbo.tile([80, TN], F32, tag="out_sb")
                    nc.scalar.activation(sb[:, :], ps[:, :], AF.Identity,
                                         bias=mob[:, :])
                    nc.sync.dma_start(out.ap()[:, nt * TN:(nt + 1) * TN], sb[:, :])


def _attention(tc, nc, d, te, ctx, melT, S, T, n, NS, NT, SN, TN):
    n_tn = NT // TN
    n_sc = NS // 128
    with tc.tile_pool(name="aw", bufs=1) as awp, \
         tc.tile_pool(name="aps", bufs=4, space="PSUM") as pa, \
         tc.tile_pool(name="asb", bufs=4) as asb, \
         tc.tile_pool(name="aqt", bufs=1) as qtp, \
         tc.tile_pool(name="akt", bufs=1) as ktp, \
         tc.tile_pool(name="avs", bufs=1) as vsp, \
         tc.tile_pool(name="aet", bufs=2) as etp, \
         tc.tile_pool(name="actx", bufs=3) as cxp:

        wk = _load(nc, awp, d, "wk", (128, 2048), BF16)
        wv = _load(nc, awp, d, "wv", (128, 2048), BF16)
        vb = _load(nc, awp, d, "vb", (64, 8), F32)
        qw = _load(nc, awp, d, "qw", (81, 512), BF16)
        ones = awp.tile([1, 64], F32, tag="ones64")
        nc.vector.memset(ones[:, :], 1.0)

        # QT sbuf-resident: [128, 4mc x NT]
        qt = qtp.tile([128, 4 * NT], BF16, tag="qt")
        for mc in range(4):
            for nt in range(n_tn):
                ps = pa.tile([128, TN], F32, tag="ps")
                nc.tensor.matmul(ps[:, :], qw[:, mc * 128:(mc + 1) * 128],
                                 melT[:, nt * TN:(nt + 1) * TN],
                                 start=True, stop=True)
                _plain_copy(nc, mc * n_tn + nt,
                            qt[:, mc * NT + nt * TN:mc * NT + (nt + 1) * TN],
                            ps[:, :])

        # KT sbuf-resident: [128, 4mc x NS]
        kt = ktp.tile([128, 4 * NS], BF16, tag="kt")
        for mc in range(4):
            for nt in range(NS // SN):
                ps = pa.tile([128, SN], F32, tag="ps")
                for kc in range(4):
                    nc.tensor.matmul(
                        ps[:, :], wk[:, (kc * 4 + mc) * 128:(kc * 4 + mc + 1) * 128],
                        te[:, kc * NS + nt * SN:kc * NS + (nt + 1) * SN],
                        start=(kc == 0), stop=(kc == 3))
                _plain_copy(nc, mc * (NS // SN) + nt,
                            kt[:, mc * NS + nt * SN:mc * NS + (nt + 1) * SN],
                            ps[:, :])

        # V with ones column per head: [128(s-sub), n_sc x (8h x 65)]
        vsb = vsp.tile([128, n_sc * 520], BF16, tag="vsb")
        for sc in range(n_sc):
            ps = pa.tile([128, 512], F32, tag="ps")
            for kc in range(4):
                nc.tensor.matmul(
                    ps[:, :], te[:, kc * NS + sc * 128:kc * NS + sc * 128 + 128],
                    wv[:, kc * 512:(kc + 1) * 512],
                    start=(kc == 0), stop=(kc == 3))
            dst = vsb[:, sc * 520:(sc + 1) * 520].rearrange("p (h c) -> p h c", h=8)
            _plain_copy(nc, sc, dst[:, :, 0:64],
                        ps[:, :].rearrange("p (h c) -> p h c", h=8))
            nc.vector.memset(dst[:, :, 64:65], 1.0)

        # per (example, head): scoresT -> exp -> ctx + sums -> scale
        nsc_b = S // 128  # s-chunks per example
        for b in range(n):
            for h in range(NH):
                hc, hr = h // 2, (h % 2) * 64
                et = etp.tile([128, nsc_b * T], BF16, tag="et")
                for tt in range(T // TN):
                    qs = qt[hr:hr + 64,
                            hc * NT + b * T + tt * TN:hc * NT + b * T + (tt + 1) * TN]
                    for scl in range(nsc_b):
                        ps = pa.tile([128, TN], F32, tag="ps")
                        nc.tensor.matmul(
                            ps[:, :],
                            kt[hr:hr + 64,
                               hc * NS + b * S + scl * 128:hc * NS + b * S + scl * 128 + 128],
                            qs, start=True, stop=True)
                        nc.scalar.activation(
                            et[:, scl * T + tt * TN:scl * T + (tt + 1) * TN],
                            ps[:, :], AF.Exp)
                cps = [pa.tile([65, TN], F32, tag="ps", name=f"cps{tt}")
                       for tt in range(T // TN)]
                for scl in range(nsc_b):
                    lhs = vsb[:, (b * nsc_b + scl) * 520 + h * 65:
                              (b * nsc_b + scl) * 520 + (h + 1) * 65]
                    for tt in range(T // TN):
                        nc.tensor.matmul(cps[tt][:, :], lhs,
                                         et[:, scl * T + tt * TN:scl * T + (tt + 1) * TN],
                                         start=(scl == 0), stop=(scl == nsc_b - 1))
                for tt in range(T // TN):
                    rc = cxp.tile([1, TN], F32, tag="recip")
                    nc.vector.reciprocal(rc[:, :], cps[tt][64:65, :])
                    rb = pa.tile([64, TN], F32, tag="ps")
                    nc.tensor.matmul(rb[:, :], ones[:, :], rc[:, :],
                                     start=True, stop=True)
                    c0 = cxp.tile([64, TN], F32, tag="ctx_unsc")
                    _plain_copy(nc, b * NH + h + tt, c0[:, :], cps[tt][0:64, :])
                    sc1 = cxp.tile([64, TN], F32, tag="ctx_scaled")
                    nc.vector.tensor_mul(sc1[:, :], c0[:, :], rb[:, :])
                    nc.vector.tensor_scalar_add(
                        ctx[hr:hr + 64,
                            hc * NT + b * T + tt * TN:hc * NT + b * T + (tt + 1) * TN],
                        sc1[:, :], vb[:, h:h + 1])


# ---------------------------------------------------------------------------
# host wrapper
# ---------------------------------------------------------------------------

def prep_host(inputs, S, T, n_per_core, ncores):
    f32 = np.float32
    emb = np.asarray(inputs["emb"], f32)
    idx = np.asarray(inputs["phoneme_indices"]).astype(np.int64)
    mel = np.asarray(inputs["mel_specs"], f32)

    bias_f = np.asarray(inputs["enc_bih_f"], f32) + np.asarray(inputs["enc_bhh_f"], f32)
    bias_b = np.asarray(inputs["enc_bih_b"], f32) + np.asarray(inputs["enc_bhh_b"], f32)
    tab_f = emb @ np.asarray(inputs["enc_Wih_f"], f32).T + bias_f
    tab_b = emb @ np.asarray(inputs["enc_Wih_b"], f32).T + bias_b

    tproj_W = np.asarray(inputs["tproj_W"], f32)
    tproj_b = np.asarray(inputs["tproj_b"], f32)
    Wq, Wk, Wv = np.split(np.asarray(inputs["attn_in_W"], f32), 3, axis=0)
    bq, bk, bv = np.split(np.asarray(inputs["attn_in_b"], f32), 3)
    mpw = np.asarray(inputs["mproj_in_W"], f32)
    mpb = np.asarray(inputs["mproj_in_b"], f32)
    aow = np.asarray(inputs["attn_out_W"], f32)
    aob = np.asarray(inputs["attn_out_b"], f32)
    dWih = np.asarray(inputs["dec_Wih"], f32)
    dbias = np.asarray(inputs["dec_bih"], f32) + np.asarray(inputs["dec_bhh"], f32)
    mow = np.asarray(inputs["mproj_out_W"], f32)
    mob = np.asarray(inputs["mproj_out_b"], f32)

    scale = f32(1.0) / np.sqrt(f32(HD))
    QW = (Wq @ mpw) * scale
    qb_ = (bq + Wq @ mpb) * scale
    vb_ = bv                       # tproj_b folded into te; bk softmax-invariant
    W1, W2 = dWih[:, :H], dWih[:, H:]
    M1 = W1 @ mpw
    M2 = W2 @ aow
    dbias_ = dbias + W1 @ mpb + W2 @ aob

    def bf(a):
        return np.ascontiguousarray(a.astype(BF))

    common = {
        "tab_f": bf(np.concatenate([tab_f[:128], tab_f[128:]], axis=1)),
        "tab_b": bf(np.concatenate([tab_b[:128], tab_b[128:]], axis=1)),
        "whh_f": bf(_lhsT_tiles(np.asarray(inputs["enc_Whh_f"], f32).T)),
        "whh_b": bf(_lhsT_tiles(np.asarray(inputs["enc_Whh_b"], f32).T)),
        "whh_d": bf(_lhsT_tiles(np.asarray(inputs["dec_Whh"], f32).T)),
        "wt_f": bf(_lhsT_tiles(tproj_W[:, :H].T)),
        "wt_b": bf(_lhsT_tiles(tproj_W[:, H:].T)),
        "tb": _bias_tiles(tproj_b),
        "wk": bf(_lhsT_tiles(Wk.T)),
        "wv": bf(Wv.T.reshape(4, 128, 512).transpose(1, 0, 2).reshape(128, 2048)),
        "vb": np.ascontiguousarray(vb_.reshape(8, 64).T.astype(f32)),
        "qw": bf(np.concatenate([QW.T, qb_.reshape(1, 512)], axis=0)),
        "m1": bf(np.concatenate([M1.T, dbias_.reshape(1, 2048)], axis=0)),
        "m2": bf(M2.T.reshape(4, 128, 2048).transpose(1, 0, 2).reshape(128, 4 * 2048)),
        "mo": bf(mow.T.reshape(4, 128, 80).transpose(1, 0, 2).reshape(128, 320)),
        "mob": np.ascontiguousarray(mob.reshape(80, 1).astype(f32)),
    }

    shifted = np.concatenate([np.zeros_like(mel[:, :1]), mel[:, :-1]], axis=1)

    in_maps = []
    for c in range(ncores):
        exs = list(range(c * n_per_core, (c + 1) * n_per_core))
        ohf = np.zeros((VOCAB, n_per_core * S), f32)
        ohb = np.zeros((VOCAB, n_per_core * S), f32)
        cols = np.arange(S)
        for bi, e in enumerate(exs):
            ohf[idx[e, :S], bi * S + cols] = 1.0
            ohb[idx[e, S - 1 - cols], bi * S + cols] = 1.0
        melTc = np.ones((MEL + 1, n_per_core * T), f32)
        for bi, e in enumerate(exs):
            melTc[:MEL, bi * T:(bi + 1) * T] = shifted[e, :T].T
        m = dict(common)
        m["oh_f"] = bf(np.concatenate([ohf[:128], ohf[128:]], axis=1))
        m["oh_b"] = bf(np.concatenate([ohb[:128], ohb[128:]], axis=1))
        m["melT"] = bf(melTc)
        in_maps.append(m)
    return in_maps


def run(inputs, S, T, n, ncores, trace=False):
    nc = build_program(S=S, T=T, n=n)
    in_maps = prep_host(inputs, S, T, n, ncores)
    res = bass_utils.run_bass_kernel_spmd(
        nc, in_maps, core_ids=list(range(ncores)), trace=trace)
    Bt = n * ncores
    out = np.zeros((Bt, T, MEL), np.float32)
    for c in range(ncores):
        o = np.asarray(res.results[c]["out"])
        for bi in range(n):
            out[c * n + bi] = o[:, bi * T:(bi + 1) * T].T
    return out, res


def kernel(**inputs):
    out, _ = run(inputs, S_FULL, T_FULL, B_FULL // NCORES, NCORES)
    return out



# revision 3
# speedup vs baseline: 6.2950x; 6.2950x over previous
# Trainium2 Bass kernel for nn_KokoroModel (text->mel seq2seq, LSTM enc/dec + MHA).
#
# Sharding: data-parallel over batch, 4 examples/core on 8 cores, weights
# replicated. Host-side weight-only fusions (model reparametrizations):
#   TAB  = emb @ enc_Wih.T            (embedding gather -> one-hot matmul)
#   QW   = (Wq @ mproj_in_W) / 8      (mel proj + Q proj + attn scale)
#   M1   = dec_Wih[:, :H] @ mproj_in_W
#   M2   = dec_Wih[:, H:] @ attn_out_W
#   K/V biases absorb tproj_b; decoder gate bias absorbs mproj_in/attn_out biases.
# Softmax normalization is deferred: exp-scores stay unnormalized, row-sums come
# from a ones-column appended to V, context is scaled by 1/sum afterwards.
#
# LSTM recurrence design (the serial bottleneck, 2024 steps):
#  - Gate types go to four separate PSUM banks so ACT can read a finished
#    bank while PE writes the next (PSUM collisions are bank-level). PE
#    emission order is [g, i, f, o]: the c-update chain (needs g,i,f) hides
#    under the o-bank matmuls; only sigmoid(o) + final h-mul trail the stream.
#  - xg (input gates) are precomputed to DRAM laid out [128,(type,gc,b,t)]
#    (t contiguous) and streamed in PF-step slabs, prefetched one slab ahead;
#    an identity-matmul accumulates xg into each PSUM bank (no DVE add).
#  - h_t is written by the final DVE mul directly (strided) into the SBUF
#    h-history buffer in (kc, b, t) layout; next-step matmuls read strided
#    columns from there. No per-step DMA.
#  - Encoder fwd/bwd chains interleave per tick: one direction's gate algebra
#    overlaps the other's matmul stream.

import numpy as np
import ml_dtypes

import concourse.bass as bass
import concourse.tile as tile
from concourse import bacc, mybir
from concourse import bass_utils

F32 = mybir.dt.float32
BF16 = mybir.dt.bfloat16
AF = mybir.ActivationFunctionType
BF = ml_dtypes.bfloat16

VOCAB, MEL, H = 256, 80, 512
NH, HD = 8, 64
B_FULL, S_FULL, T_FULL = 32, 512, 1000
NCORES = 8
PF = 16  # xg slab length (steps per prefetch)

# gate-type permutation: torch order [i,f,g,o] -> ours [i,f,o,g] (type-major)
GPERM = np.r_[0:H, H:2 * H, 3 * H:4 * H, 2 * H:3 * H]


def _lhsT_tiles(w, kp=128, mp=128):
    """w: [K, M] -> [kp, (K//kp)*(M//mp)*mp]; tile (kc,mc) at cols
    (kc*(M//mp)+mc)*mp."""
    K, M = w.shape
    nk, nm = K // kp, M // mp
    return np.ascontiguousarray(
        w.reshape(nk, kp, nm, mp).transpose(1, 0, 2, 3).reshape(kp, nk * nm * mp))


# ---------------------------------------------------------------------------
# device program
# ---------------------------------------------------------------------------

def build_program(S=S_FULL, T=T_FULL, n=B_FULL // NCORES, stages=5):
    nc = bacc.Bacc("TRN2", target_bir_lowering=False, debug=False)

    NS, NT = n * S, n * T
    TN = T // 2       # mel matmul N-tile
    assert TN <= 512

    d = {}

    def din(name, shape, dt):
        d[name] = nc.dram_tensor(name, list(shape), dt, kind="ExternalInput")

    din("ident", (128, 128), BF16)
    din("oh_f", (128, 2 * NS), BF16)
    din("oh_b", (128, 2 * NS), BF16)
    din("tab_f", (128, 2 * 16 * 128), BF16)   # stationary tiles (vc, gc)
    din("tab_b", (128, 2 * 16 * 128), BF16)
    din("whh_f", (128, 4 * 16 * 128), BF16)   # stationary tiles (kc, gc)
    din("whh_b", (128, 4 * 16 * 128), BF16)
    din("whh_d", (128, 4 * 16 * 128), BF16)
    din("wt_f", (128, 4 * 4 * 128), BF16)
    din("wt_b", (128, 4 * 4 * 128), BF16)
    din("tb", (128, 4), F32)
    din("wk", (128, 4 * 4 * 128), BF16)
    din("wv", (128, 4 * 512), BF16)
    din("vb", (64, 8), F32)
    din("qw", (81, 4 * 128), BF16)
    din("melT", (81, NT), BF16)
    din("m1", (81, 16 * 128), BF16)           # stationary tiles (gc)
    din("m2", (128, 4 * 16 * 128), BF16)      # stationary tiles (hc, gc)
    din("mo", (128, 4 * 80), BF16)
    din("mob", (80, 1), F32)

    out = nc.dram_tensor("out", [80, NT], F32, kind="ExternalOutput")

    # xg layout: [128, (type4*gc4, b, t)] -- t contiguous per (type,gc,b)
    xg_f = nc.dram_tensor("xg_f", [128, 16 * NS], BF16, kind="Internal")
    xg_b = nc.dram_tensor("xg_b", [128, 16 * NS], BF16, kind="Internal")
    xg_d = nc.dram_tensor("xg_d", [128, 16 * NT], BF16, kind="Internal")

    with tile.TileContext(nc) as tc:
        _body(tc, nc, d, out, xg_f, xg_b, xg_d, S, T, n, NS, NT, TN, stages)

    nc.compile()
    return nc


def _plain_copy(nc, i, dst, src):
    if i % 2 == 0:
        nc.scalar.copy(dst, src)
    else:
        nc.vector.tensor_copy(dst, src)


def _load(nc, pool, d, name, shape, dt):
    t_ = pool.tile(list(shape), dt, tag=name)
    nc.sync.dma_start(t_[:, :], d[name].ap())
    return t_


def _xg_produce(tc, nc, psp, sbp, tab, oh, xg_dram, S, n):
    """xg[p, (g, b, t)] = (TAB_tile.T @ onehot)[gate, token]; bias folded in.
    Gates land on partitions; DRAM runs are contiguous in t."""
    NS = n * S
    TB = min(S, 512)
    nsb = S // TB
    xg_ap = xg_dram.ap().rearrange("p (g b t) -> p g b t", g=16, b=n)
    i = 0
    for b in range(n):
        st = sbp.tile([128, 16 * S], BF16, tag="xg_stage")
        sv = st[:, :].rearrange("p (g t) -> p g t", g=16)
        for gc in range(16):
            for tb in range(nsb):
                ps = psp.tile([128, 512], F32, tag="ps")
                col0 = b * S + tb * TB
                for vc in range(2):
                    nc.tensor.matmul(
                        ps[:, :TB],
                        tab[:, (vc * 16 + gc) * 128:(vc * 16 + gc + 1) * 128],
                        oh[:, vc * NS + col0:vc * NS + col0 + TB],
                        start=(vc == 0), stop=(vc == 1))
                _plain_copy(nc, i, sv[:, gc, tb * TB:(tb + 1) * TB], ps[:, :TB])
                i += 1
        nc.sync.dma_start(xg_ap[:, :, b, :], sv[:, :, :])


class _LstmChain:
    """State + per-step emission for one LSTM recurrence."""

    def __init__(self, nc, pools, T_steps, n, xg_dram, whh, ob, col_of_t,
                 ident, tag):
        self.nc = nc
        (self.psp, self.stp, self.actp, self.prodp, self.tcp, self.xgp) = pools
        self.T_steps, self.n = T_steps, n
        self.xg_ap = xg_dram.ap().rearrange("p (g b t) -> p g b t", g=16, b=n)
        self.whh = whh
        self.ob = ob.rearrange("p (kc b t) -> p kc b t", kc=4, b=n)
        self.col_of_t = col_of_t
        self.ident = ident
        self.tag = tag
        self.gw = 4 * n
        self.pending = []
        st = self.stp.tile([128, 2 * self.gw], F32, tag=f"st_{tag}")
        nc.vector.memset(st[:, self.gw:2 * self.gw], 0.0)   # c = 0
        self.st = st
        self.xgs = None

    def prefetch(self, t0):
        if t0 >= self.T_steps:
            return
        pf = min(PF, self.T_steps - t0)
        xgs = self.xgp.tile([128, 16 * self.n * PF], BF16,
                            tag=f"xgs_{self.tag}")
        xv = xgs[:, :].rearrange("p (g b t) -> p g b t", g=16, t=PF)
        self.nc.sync.dma_start(xv[:, :, :, :pf], self.xg_ap[:, :, :, t0:t0 + pf])
        self.pending.append(xgs)

    def step(self, t):
        nc, n, gw = self.nc, self.n, self.gw
        if t % PF == 0:
            self.xgs = self.pending.pop(0)
            self.prefetch(t + PF)
        toff = t % PF
        xv = self.xgs[:, :].rearrange("p (g b t) -> p g b t", g=16, t=PF)

        # PE bank order [g, i, f, o]; type indices: i=0, f=1, o=2, g=3.
        for typ in (3, 0, 1, 2):
            ps = self.psp.tile([128, 512], F32, tag=f"ps{typ}_{self.tag}")
            pv = ps[:, 0:gw].rearrange("p (g b) -> p g b", g=4)
            # identity-matmul accumulates xg into the bank (start clears it)
            nc.tensor.matmul(
                pv[:, :, :], self.ident[:, :],
                xv[:, typ * 4:(typ + 1) * 4, :, toff],
                start=True, stop=(t == 0))
            if t > 0:
                hcol = self.col_of_t(t - 1)
                for gcl in range(4):
                    gg = typ * 4 + gcl
                    for kc in range(4):
                        nc.tensor.matmul(
                            ps[:, gcl * n:(gcl + 1) * n],
                            self.whh[:, (kc * 16 + gg) * 128:
                                     (kc * 16 + gg + 1) * 128],
                            self.ob[:, kc, :, hcol],
                            start=False, stop=(kc == 3))
            # ALG, interleaved so each bank is consumed while PE moves on
            if typ == 3:      # tanh(g) -> st[0:gw]
                nc.scalar.activation(self.st[:, 0:gw], ps[:, 0:gw], AF.Tanh)
                self.acts = self.actp.tile([128, 3 * gw], F32,
                                           tag=f"acts_{self.tag}")
            elif typ == 0:    # sigmoid(i) -> acts[0:gw]
                nc.scalar.activation(self.acts[:, 0:gw], ps[:, 0:gw],
                                     AF.Sigmoid)
            elif typ == 1:    # sigmoid(f); then c-chain (independent of o)
                nc.scalar.activation(self.acts[:, gw:2 * gw], ps[:, 0:gw],
                                     AF.Sigmoid)
                prod = self.prodp.tile([128, 2 * gw], F32,
                                       tag=f"prod_{self.tag}")
                nc.vector.tensor_mul(prod[:, :], self.acts[:, 0:2 * gw],
                                     self.st[:, :])
                nc.vector.tensor_add(self.st[:, gw:2 * gw], prod[:, 0:gw],
                                     prod[:, gw:2 * gw])
                tc_ = self.tcp.tile([128, gw], F32, tag=f"tc_{self.tag}")
                nc.scalar.activation(tc_[:, :], self.st[:, gw:2 * gw], AF.Tanh)
                self.tc_ = tc_
            else:             # sigmoid(o); h = sig(o) * tanh(c) -> ob[..., t]
                nc.scalar.activation(self.acts[:, 2 * gw:3 * gw], ps[:, 0:gw],
                                     AF.Sigmoid)
                nc.vector.tensor_mul(
                    self.ob[:, :, :, self.col_of_t(t)],
                    self.acts[:, 2 * gw:3 * gw].rearrange(
                        "p (kc b) -> p kc b", kc=4),
                    self.tc_[:, :].rearrange("p (kc b) -> p kc b", kc=4))


def _lstm_phase(chains, T_steps):
    for ch in chains:
        ch.prefetch(0)
    for t in range(T_steps):
        for ch in chains:
            ch.step(t)


def _body(tc, nc, d, out, xg_f, xg_b, xg_d, S, T, n, NS, NT, TN, stages=5):
    n_tn = NT // TN

    with tc.tile_pool(name="persist", bufs=1) as pp:
        te = pp.tile([128, 4 * NS], BF16, tag="te")
        ctx = pp.tile([128, 4 * NT], BF16, tag="ctx")
        ident = _load(nc, pp, d, "ident", (128, 128), BF16)

        # ---------------- phase E0: encoder gate precompute ----------------
        with tc.tile_pool(name="e0w", bufs=1) as wp, \
             tc.tile_pool(name="e0ps", bufs=4, space="PSUM") as psp, \
             tc.tile_pool(name="e0sb", bufs=2) as sbp:
            tab_f = _load(nc, wp, d, "tab_f", (128, 4096), BF16)
            tab_b = _load(nc, wp, d, "tab_b", (128, 4096), BF16)
            oh_f = _load(nc, wp, d, "oh_f", (128, 2 * NS), BF16)
            oh_b = _load(nc, wp, d, "oh_b", (128, 2 * NS), BF16)
            _xg_produce(tc, nc, psp, sbp, tab_f, oh_f, xg_f, S, n)
            _xg_produce(tc, nc, psp, sbp, tab_b, oh_b, xg_b, S, n)
        if stages <= 1:
            return

        # ---------------- phase E1: encoder recurrences (interleaved) ------
        with tc.tile_pool(name="e1w", bufs=1) as ewp, \
             tc.tile_pool(name="e1buf", bufs=1) as ebp:
            whh_f = _load(nc, ewp, d, "whh_f", (128, 8192), BF16)
            whh_b = _load(nc, ewp, d, "whh_b", (128, 8192), BF16)
            buf_f = ebp.tile([128, 4 * NS], BF16, tag="buf_f")
            buf_b = ebp.tile([128, 4 * NS], BF16, tag="buf_b")

            with tc.tile_pool(name="e1ps", bufs=1, space="PSUM") as pls, \
                 tc.tile_pool(name="e1st", bufs=1) as stp, \
                 tc.tile_pool(name="e1act", bufs=2) as actp, \
                 tc.tile_pool(name="e1prod", bufs=2) as prodp, \
                 tc.tile_pool(name="e1tc", bufs=2) as tcp, \
                 tc.tile_pool(name="e1xg", bufs=3) as xgp:
                lp = (pls, stp, actp, prodp, tcp, xgp)
                chf = _LstmChain(nc, lp, S, n, xg_f, whh_f, buf_f[:, :],
                                 lambda t: t, ident, "f")
                chb = _LstmChain(nc, lp, S, n, xg_b, whh_b, buf_b[:, :],
                                 lambda t: S - 1 - t, ident, "b")
                _lstm_phase([chf, chb], S)

            # ---------------- phase E2: text projection -------------------
            with tc.tile_pool(name="e2w", bufs=1) as wtp, \
                 tc.tile_pool(name="e2ps", bufs=4, space="PSUM") as ptp:
                wt_f = _load(nc, wtp, d, "wt_f", (128, 2048), BF16)
                wt_b = _load(nc, wtp, d, "wt_b", (128, 2048), BF16)
                tb = _load(nc, wtp, d, "tb", (128, 4), F32)
                for mc in range(4):
                    for b in range(n):
                        ps = ptp.tile([128, 512], F32, tag="ps")
                        for kc in range(4):
                            nc.tensor.matmul(
                                ps[:, :S],
                                wt_f[:, (kc * 4 + mc) * 128:(kc * 4 + mc + 1) * 128],
                                buf_f[:, kc * NS + b * S:kc * NS + (b + 1) * S],
                                start=(kc == 0), stop=False)
                        for kc in range(4):
                            nc.tensor.matmul(
                                ps[:, :S],
                                wt_b[:, (kc * 4 + mc) * 128:(kc * 4 + mc + 1) * 128],
                                buf_b[:, kc * NS + b * S:kc * NS + (b + 1) * S],
                                start=False, stop=(kc == 3))
                        dst = te[:, mc * NS + b * S:mc * NS + (b + 1) * S]
                        if (mc * n + b) % 2 == 0:
                            nc.scalar.activation(dst, ps[:, :S], AF.Identity,
                                                 bias=tb[:, mc:mc + 1])
                        else:
                            nc.vector.tensor_scalar_add(dst, ps[:, :S],
                                                        tb[:, mc:mc + 1])

        if stages <= 2:
            return
        # ---------------- phase A: attention ------------------------------
        melT = pp.tile([81, NT], BF16, tag="melT")
        nc.sync.dma_start(melT[:, :], d["melT"].ap())
        _attention(tc, nc, d, te, ctx, melT, S, T, n, NS, NT, TN)

        if stages <= 3:
            return
        # ---------------- phase X: decoder gate precompute ----------------
        with tc.tile_pool(name="xw", bufs=1) as dwp, \
             tc.tile_pool(name="xps", bufs=4, space="PSUM") as pdx, \
             tc.tile_pool(name="xsb", bufs=2) as dsb:
            m1 = _load(nc, dwp, d, "m1", (81, 2048), BF16)
            m2 = _load(nc, dwp, d, "m2", (128, 4 * 2048), BF16)
            xd_ap = xg_d.ap().rearrange("p (g b t) -> p g b t", g=16, b=n)
            i = 0
            for b in range(n):
                for tbk in range(T // TN):
                    st = dsb.tile([128, 16 * TN], BF16, tag="xd_stage")
                    sv = st[:, :].rearrange("p (g t) -> p g t", g=16)
                    for gc in range(16):
                        ps = pdx.tile([128, 512], F32, tag="ps")
                        nc.tensor.matmul(
                            ps[:, :TN], m1[:, gc * 128:(gc + 1) * 128],
                            melT[:, b * T + tbk * TN:b * T + (tbk + 1) * TN],
                            start=True, stop=False)
                        for hc in range(4):
                            nc.tensor.matmul(
                                ps[:, :TN],
                                m2[:, (hc * 16 + gc) * 128:(hc * 16 + gc + 1) * 128],
                                ctx[:, hc * NT + b * T + tbk * TN:
                                    hc * NT + b * T + (tbk + 1) * TN],
                                start=False, stop=(hc == 3))
                        _plain_copy(nc, i, sv[:, gc, :], ps[:, :TN])
                        i += 1
                    nc.sync.dma_start(xd_ap[:, :, b, tbk * TN:(tbk + 1) * TN],
                                      sv[:, :, :])

        if stages <= 4:
            return
        # ---------------- phase D: decoder recurrence + out proj ----------
        with tc.tile_pool(name="dw", bufs=1) as dwp, \
             tc.tile_pool(name="dbuf", bufs=1) as dbp:
            whh_d = _load(nc, dwp, d, "whh_d", (128, 8192), BF16)
            dbuf = dbp.tile([128, 4 * NT], BF16, tag="dbuf")

            with tc.tile_pool(name="dps", bufs=2, space="PSUM") as pls, \
                 tc.tile_pool(name="dst", bufs=1) as stp, \
                 tc.tile_pool(name="dact", bufs=2) as actp, \
                 tc.tile_pool(name="dprod", bufs=2) as prodp, \
                 tc.tile_pool(name="dtc", bufs=2) as tcp, \
                 tc.tile_pool(name="dxg", bufs=3) as xgp:
                lp = (pls, stp, actp, prodp, tcp, xgp)
                chd = _LstmChain(nc, lp, T, n, xg_d, whh_d, dbuf[:, :],
                                 lambda t: t, ident, "d")
                _lstm_phase([chd], T)

            with tc.tile_pool(name="ow", bufs=1) as mop, \
                 tc.tile_pool(name="ops", bufs=4, space="PSUM") as pso, \
                 tc.tile_pool(name="osb", bufs=4) as sbo:
                mo = _load(nc, mop, d, "mo", (128, 320), BF16)
                mob = _load(nc, mop, d, "mob", (80, 1), F32)
                for nt in range(n_tn):
                    ps = pso.tile([80, TN], F32, tag="ps")
                    for kc in range(4):
                        nc.tensor.matmul(
                            ps[:, :], mo[:, kc * 80:(kc + 1) * 80],
                            dbuf[:, kc * NT + nt * TN:kc * NT + (nt + 1) * TN],
                            start=(kc == 0), stop=(kc == 3))
                    sb = sbo.tile([80, TN], F32, tag="out_sb")
                    nc.scalar.activation(sb[:, :], ps[:, :], AF.Identity,
                                         bias=mob[:, :])
                    nc.sync.dma_start(out.ap()[:, nt * TN:(nt + 1) * TN],
                                      sb[:, :])


def _attention(tc, nc, d, te, ctx, melT, S, T, n, NS, NT, TN):
    n_tn = NT // TN
    n_sc = NS // 128
    with tc.tile_pool(name="aw", bufs=1) as awp, \
         tc.tile_pool(name="aps", bufs=4, space="PSUM") as pa, \
         tc.tile_pool(name="aqt", bufs=1) as qtp, \
         tc.tile_pool(name="akt", bufs=1) as ktp, \
         tc.tile_pool(name="avs", bufs=1) as vsp, \
         tc.tile_pool(name="aet", bufs=2) as etp, \
         tc.tile_pool(name="actx", bufs=3) as cxp:

        wk = _load(nc, awp, d, "wk", (128, 2048), BF16)
        wv = _load(nc, awp, d, "wv", (128, 2048), BF16)
        vb = _load(nc, awp, d, "vb", (64, 8), F32)
        qw = _load(nc, awp, d, "qw", (81, 512), BF16)
        ones = awp.tile([1, 64], F32, tag="ones64")
        nc.vector.memset(ones[:, :], 1.0)

        # QT sbuf-resident: [128, 4mc x NT]
        qt = qtp.tile([128, 4 * NT], BF16, tag="qt")
        for mc in range(4):
            for nt in range(n_tn):
                ps = pa.tile([128, TN], F32, tag="ps")
                nc.tensor.matmul(ps[:, :], qw[:, mc * 128:(mc + 1) * 128],
                                 melT[:, nt * TN:(nt + 1) * TN],
                                 start=True, stop=True)
                _plain_copy(nc, mc * n_tn + nt,
                            qt[:, mc * NT + nt * TN:mc * NT + (nt + 1) * TN],
                            ps[:, :])

        # KT sbuf-resident: [128, 4mc x NS]
        kt = ktp.tile([128, 4 * NS], BF16, tag="kt")
        for mc in range(4):
            for b in range(n):
                ps = pa.tile([128, 512], F32, tag="ps")
                for kc in range(4):
                    nc.tensor.matmul(
                        ps[:, :S],
                        wk[:, (kc * 4 + mc) * 128:(kc * 4 + mc + 1) * 128],
                        te[:, kc * NS + b * S:kc * NS + (b + 1) * S],
                        start=(kc == 0), stop=(kc == 3))
                _plain_copy(nc, mc * n + b,
                            kt[:, mc * NS + b * S:mc * NS + (b + 1) * S],
                            ps[:, :S])

        # V with ones column per head: [128(s-sub), n_sc x (8h x 65)]
        vsb = vsp.tile([128, n_sc * 520], BF16, tag="vsb")
        for sc in range(n_sc):
            ps = pa.tile([128, 512], F32, tag="ps")
            for kc in range(4):
                nc.tensor.matmul(
                    ps[:, :], te[:, kc * NS + sc * 128:kc * NS + sc * 128 + 128],
                    wv[:, kc * 512:(kc + 1) * 512],
                    start=(kc == 0), stop=(kc == 3))
            dst = vsb[:, sc * 520:(sc + 1) * 520].rearrange("p (h c) -> p h c", h=8)
            _plain_copy(nc, sc, dst[:, :, 0:64],
                        ps[:, :].rearrange("p (h c) -> p h c", h=8))
            nc.vector.memset(dst[:, :, 64:65], 1.0)

        # per (example, head): scoresT -> exp -> ctx + sums -> scale
        nsc_b = S // 128  # s-chunks per example
        for b in range(n):
            for h in range(NH):
                hc, hr = h // 2, (h % 2) * 64
                et = etp.tile([128, nsc_b * T], BF16, tag="et")
                for tt in range(T // TN):
                    qs = qt[hr:hr + 64,
                            hc * NT + b * T + tt * TN:hc * NT + b * T + (tt + 1) * TN]
                    for scl in range(nsc_b):
                        ps = pa.tile([128, TN], F32, tag="ps")
                        nc.tensor.matmul(
                            ps[:, :],
                            kt[hr:hr + 64,
                               hc * NS + b * S + scl * 128:hc * NS + b * S + scl * 128 + 128],
                            qs, start=True, stop=True)
                        nc.scalar.activation(
                            et[:, scl * T + tt * TN:scl * T + (tt + 1) * TN],
                            ps[:, :], AF.Exp)
                cps = [pa.tile([65, TN], F32, tag="ps", name=f"cps{tt}")
                       for tt in range(T // TN)]
                for scl in range(nsc_b):
                    lhs = vsb[:, (b * nsc_b + scl) * 520 + h * 65:
                              (b * nsc_b + scl) * 520 + (h + 1) * 65]
                    for tt in range(T // TN):
                        nc.tensor.matmul(cps[tt][:, :], lhs,
                                         et[:, scl * T + tt * TN:scl * T + (tt + 1) * TN],
                                         start=(scl == 0), stop=(scl == nsc_b - 1))
                for tt in range(T // TN):
                    rc = cxp.tile([1, TN], F32, tag="recip")
                    nc.vector.reciprocal(rc[:, :], cps[tt][64:65, :])
                    rb = pa.tile([64, TN], F32, tag="ps")
                    nc.tensor.matmul(rb[:, :], ones[:, :], rc[:, :],
                                     start=True, stop=True)
                    c0 = cxp.tile([64, TN], F32, tag="ctx_unsc")
                    _plain_copy(nc, b * NH + h + tt, c0[:, :], cps[tt][0:64, :])
                    sc1 = cxp.tile([64, TN], F32, tag="ctx_scaled")
                    nc.vector.tensor_mul(sc1[:, :], c0[:, :], rb[:, :])
                    nc.vector.tensor_scalar_add(
                        ctx[hr:hr + 64,
                            hc * NT + b * T + tt * TN:hc * NT + b * T + (tt + 1) * TN],
                        sc1[:, :], vb[:, h:h + 1])


# ---------------------------------------------------------------------------
# host wrapper
# ---------------------------------------------------------------------------

def prep_host(inputs, S, T, n_per_core, ncores):
    f32 = np.float32
    emb = np.asarray(inputs["emb"], f32)
    idx = np.asarray(inputs["phoneme_indices"]).astype(np.int64)
    mel = np.asarray(inputs["mel_specs"], f32)

    bias_f = np.asarray(inputs["enc_bih_f"], f32) + np.asarray(inputs["enc_bhh_f"], f32)
    bias_b = np.asarray(inputs["enc_bih_b"], f32) + np.asarray(inputs["enc_bhh_b"], f32)
    tab_f = (emb @ np.asarray(inputs["enc_Wih_f"], f32).T + bias_f)[:, GPERM]
    tab_b = (emb @ np.asarray(inputs["enc_Wih_b"], f32).T + bias_b)[:, GPERM]

    tproj_W = np.asarray(inputs["tproj_W"], f32)
    tproj_b = np.asarray(inputs["tproj_b"], f32)
    Wq, Wk, Wv = np.split(np.asarray(inputs["attn_in_W"], f32), 3, axis=0)
    bq, bk, bv = np.split(np.asarray(inputs["attn_in_b"], f32), 3)
    mpw = np.asarray(inputs["mproj_in_W"], f32)
    mpb = np.asarray(inputs["mproj_in_b"], f32)
    aow = np.asarray(inputs["attn_out_W"], f32)
    aob = np.asarray(inputs["attn_out_b"], f32)
    dWih = np.asarray(inputs["dec_Wih"], f32)[GPERM]
    dbias = (np.asarray(inputs["dec_bih"], f32)
             + np.asarray(inputs["dec_bhh"], f32))[GPERM]
    mow = np.asarray(inputs["mproj_out_W"], f32)
    mob = np.asarray(inputs["mproj_out_b"], f32)

    scale = f32(1.0) / np.sqrt(f32(HD))
    QW = (Wq @ mpw) * scale
    qb_ = (bq + Wq @ mpb) * scale
    vb_ = bv                       # tproj_b folded into te; bk softmax-invariant
    W1, W2 = dWih[:, :H], dWih[:, H:]
    M1 = W1 @ mpw
    M2 = W2 @ aow
    dbias_ = dbias + W1 @ mpb + W2 @ aob

    whh_f_p = np.asarray(inputs["enc_Whh_f"], f32)[GPERM]
    whh_b_p = np.asarray(inputs["enc_Whh_b"], f32)[GPERM]
    whh_d_p = np.asarray(inputs["dec_Whh"], f32)[GPERM]

    def bf(a):
        return np.ascontiguousarray(a.astype(BF))

    common = {
        "ident": bf(np.eye(128, dtype=f32)),
        "tab_f": bf(_lhsT_tiles(tab_f)),
        "tab_b": bf(_lhsT_tiles(tab_b)),
        "whh_f": bf(_lhsT_tiles(whh_f_p.T)),
        "whh_b": bf(_lhsT_tiles(whh_b_p.T)),
        "whh_d": bf(_lhsT_tiles(whh_d_p.T)),
        "wt_f": bf(_lhsT_tiles(tproj_W[:, :H].T)),
        "wt_b": bf(_lhsT_tiles(tproj_W[:, H:].T)),
        "tb": np.ascontiguousarray(tproj_b.reshape(4, 128).T.astype(f32)),
        "wk": bf(_lhsT_tiles(Wk.T)),
        "wv": bf(Wv.T.reshape(4, 128, 512).transpose(1, 0, 2).reshape(128, 2048)),
        "vb": np.ascontiguousarray(vb_.reshape(8, 64).T.astype(f32)),
        "qw": bf(np.concatenate([QW.T, qb_.reshape(1, 512)], axis=0)),
        "m1": bf(np.concatenate([M1.T, dbias_.reshape(1, 2048)], axis=0)),
        "m2": bf(M2.T.reshape(4, 128, 2048).transpose(1, 0, 2).reshape(128, 4 * 2048)),
        "mo": bf(mow.T.reshape(4, 128, 80).transpose(1, 0, 2).reshape(128, 320)),
        "mob": np.ascontiguousarray(mob.reshape(80, 1).astype(f32)),
    }

    shifted = np.concatenate([np.zeros_like(mel[:, :1]), mel[:, :-1]], axis=1)

    in_maps = []
    for c in range(ncores):
        exs = list(range(c * n_per_core, (c + 1) * n_per_core))
        ohf = np.zeros((VOCAB, n_per_core * S), f32)
        ohb = np.zeros((VOCAB, n_per_core * S), f32)
        cols = np.arange(S)
        for bi, e in enumerate(exs):
            ohf[idx[e, :S], bi * S + cols] = 1.0
            ohb[idx[e, S - 1 - cols], bi * S + cols] = 1.0
        melTc = np.ones((MEL + 1, n_per_core * T), f32)
        for bi, e in enumerate(exs):
            melTc[:MEL, bi * T:(bi + 1) * T] = shifted[e, :T].T
        m = dict(common)
        m["oh_f"] = bf(np.concatenate([ohf[:128], ohf[128:]], axis=1))
        m["oh_b"] = bf(np.concatenate([ohb[:128], ohb[128:]], axis=1))
        m["melT"] = bf(melTc)
        in_maps.append(m)
    return in_maps


def run(inputs, S, T, n, ncores, trace=False):
    nc = build_program(S=S, T=T, n=n)
    in_maps = prep_host(inputs, S, T, n, ncores)
    res = bass_utils.run_bass_kernel_spmd(
        nc, in_maps, core_ids=list(range(ncores)), trace=trace)
    Bt = n * ncores
    out = np.zeros((Bt, T, MEL), np.float32)
    for c in range(ncores):
        o = np.asarray(res.results[c]["out"])
        for bi in range(n):
            out[c * n + bi] = o[:, bi * T:(bi + 1) * T].T
    return out, res


def kernel(**inputs):
    out, _ = run(inputs, S_FULL, T_FULL, B_FULL // NCORES, NCORES)
    return out


# revision 8
# speedup vs baseline: 10.6219x; 1.6873x over previous
# Trainium2 Bass kernel for nn_KokoroModel (text->mel seq2seq, LSTM enc/dec + MHA).
#
# Sharding: data-parallel over batch, 4 examples/core on 8 cores, weights
# replicated. Host-side weight-only fusions (model reparametrizations):
#   TAB  = emb @ enc_Wih.T            (embedding gather -> one-hot matmul)
#   QW   = (Wq @ mproj_in_W) / 8      (mel proj + Q proj + attn scale)
#   M1   = dec_Wih[:, :H] @ mproj_in_W
#   M2   = dec_Wih[:, H:] @ attn_out_W
#   K/V biases absorb tproj_b; decoder gate bias absorbs mproj_in/attn_out biases.
# Softmax normalization is deferred: exp-scores stay unnormalized, row-sums come
# from a ones-column appended to V, context is scaled by 1/sum afterwards.
#
# LSTM recurrence design (the serial bottleneck, 2024 steps):
#  - Gate types go to four separate PSUM banks so ACT can read a finished
#    bank while PE writes the next (PSUM collisions are bank-level). PE
#    emission order is [g, i, f, o]: the c-update chain (needs g,i,f) hides
#    under the o-bank matmuls; only sigmoid(o) + final h-mul trail the stream.
#  - xg (input gates) are precomputed to DRAM laid out [128,(type,gc,b,t)]
#    (t contiguous) and streamed in PF-step slabs, prefetched one slab ahead;
#    an identity-matmul accumulates xg into each PSUM bank (no DVE add).
#  - h_t is written by the final DVE mul directly (strided) into the SBUF
#    h-history buffer in (kc, b, t) layout; next-step matmuls read strided
#    columns from there. No per-step DMA.
#  - Encoder fwd/bwd chains interleave per tick: one direction's gate algebra
#    overlaps the other's matmul stream.

import numpy as np
import ml_dtypes

import concourse.bass as bass
import concourse.tile as tile
from concourse import bacc, mybir
from concourse import bass_utils

F32 = mybir.dt.float32
BF16 = mybir.dt.bfloat16
AF = mybir.ActivationFunctionType
BF = ml_dtypes.bfloat16

VOCAB, MEL, H = 256, 80, 512
NH, HD = 8, 64
B_FULL, S_FULL, T_FULL = 32, 512, 1000
NCORES = 8
PF = 16  # xg slab length (steps per prefetch)

# gate-type permutation: torch order [i,f,g,o] -> ours [i,f,o,g] (type-major)
GPERM = np.r_[0:H, H:2 * H, 3 * H:4 * H, 2 * H:3 * H]


def _lhsT_tiles(w, kp=128, mp=128):
    """w: [K, M] -> [kp, (K//kp)*(M//mp)*mp]; tile (kc,mc) at cols
    (kc*(M//mp)+mc)*mp."""
    K, M = w.shape
    nk, nm = K // kp, M // mp
    return np.ascontiguousarray(
        w.reshape(nk, kp, nm, mp).transpose(1, 0, 2, 3).reshape(kp, nk * nm * mp))


# ---------------------------------------------------------------------------
# device program
# ---------------------------------------------------------------------------

def build_program(S=S_FULL, T=T_FULL, n=B_FULL // NCORES, stages=5):
    nc = bacc.Bacc("TRN2", target_bir_lowering=False, debug=False)

    NS, NT = n * S, n * T
    TN = T // 2       # mel matmul N-tile
    assert TN <= 512

    d = {}

    def din(name, shape, dt):
        d[name] = nc.dram_tensor(name, list(shape), dt, kind="ExternalInput")

    din("ident", (128, 128), BF16)
    din("oh_f", (128, 2 * NS), BF16)
    din("oh_b", (128, 2 * NS), BF16)
    din("tab_f", (128, 2 * 16 * 128), BF16)   # stationary tiles (vc, gc)
    din("tab_b", (128, 2 * 16 * 128), BF16)
    din("whh_f", (128, 4 * 16 * 128), BF16)   # stationary tiles (kc, gc)
    din("whh_b", (128, 4 * 16 * 128), BF16)
    din("whh_d", (128, 4 * 16 * 128), BF16)
    din("wt_f", (128, 4 * 4 * 128), BF16)
    din("wt_b", (128, 4 * 4 * 128), BF16)
    din("tb", (128, 4), F32)
    din("wk", (128, 4 * 4 * 128), BF16)
    din("wv", (128, 4 * 512), BF16)
    din("vb", (64, 8), F32)
    din("qw", (81, 4 * 128), BF16)
    din("melT", (81, NT), BF16)
    din("m1", (81, 16 * 128), BF16)           # stationary tiles (gc)
    din("m2", (128, 4 * 16 * 128), BF16)      # stationary tiles (hc, gc)
    din("mo", (128, 4 * 80), BF16)
    din("mob", (80, 1), F32)

    out = nc.dram_tensor("out", [80, NT], F32, kind="ExternalOutput")

    # xg layout: [128, (type4*gc4, b, t)] -- t contiguous per (type,gc,b)
    xg_f = nc.dram_tensor("xg_f", [128, 16 * NS], BF16, kind="Internal")
    xg_b = nc.dram_tensor("xg_b", [128, 16 * NS], BF16, kind="Internal")
    xg_d = nc.dram_tensor("xg_d", [128, 16 * NT], BF16, kind="Internal")

    with tile.TileContext(nc) as tc:
        _body(tc, nc, d, out, xg_f, xg_b, xg_d, S, T, n, NS, NT, TN, stages)

    nc.compile()
    return nc


def _plain_copy(nc, i, dst, src):
    if i % 2 == 0:
        nc.scalar.copy(dst, src)
    else:
        nc.vector.tensor_copy(dst, src)


def _load(nc, pool, d, name, shape, dt):
    t_ = pool.tile(list(shape), dt, tag=name)
    nc.sync.dma_start(t_[:, :], d[name].ap())
    return t_


def _xg_produce(tc, nc, psp, sbp, tab, oh, xg_dram, S, n):
    """xg[p, (g, b, t)] = (TAB_tile.T @ onehot)[gate, token]; bias folded in.
    Gates land on partitions; DRAM runs are contiguous in t."""
    NS = n * S
    TB = min(S, 512)
    nsb = S // TB
    xg_ap = xg_dram.ap().rearrange("p (g b t) -> p g b t", g=16, b=n)
    i = 0
    for b in range(n):
        st = sbp.tile([128, 16 * S], BF16, tag="xg_stage")
        sv = st[:, :].rearrange("p (g t) -> p g t", g=16)
        for gc in range(16):
            for tb in range(nsb):
                ps = psp.tile([128, 512], F32, tag="ps")
                col0 = b * S + tb * TB
                for vc in range(2):
                    nc.tensor.matmul(
                        ps[:, :TB],
                        tab[:, (vc * 16 + gc) * 128:(vc * 16 + gc + 1) * 128],
                        oh[:, vc * NS + col0:vc * NS + col0 + TB],
                        start=(vc == 0), stop=(vc == 1))
                _plain_copy(nc, i, sv[:, gc, tb * TB:(tb + 1) * TB], ps[:, :TB])
                i += 1
        nc.sync.dma_start(xg_ap[:, :, b, :], sv[:, :, :])


class _LstmChain:
    """State + per-step emission for one LSTM recurrence."""

    def __init__(self, nc, pools, T_steps, n, xg_dram, whh, ob, col_of_t,
                 ident, tag):
        self.nc = nc
        (self.psp, self.stp, self.actp, self.prodp, self.tcp, self.xgp) = pools
        self.T_steps, self.n = T_steps, n
        self.xg_ap = xg_dram.ap().rearrange("p (g b t) -> p g b t", g=16, b=n)
        self.whh = whh
        self.ob = ob.rearrange("p (kc b t) -> p kc b t", kc=4, b=n)
        self.col_of_t = col_of_t
        self.ident = ident
        self.tag = tag
        self.gw = 4 * n
        self.pending = []
        st = self.stp.tile([128, 2 * self.gw], F32, tag=f"st_{tag}")
        nc.vector.memset(st[:, self.gw:2 * self.gw], 0.0)   # c = 0
        self.st = st
        self.xgs = None

    def prefetch(self, t0):
        if t0 >= self.T_steps:
            return
        pf = min(PF, self.T_steps - t0)
        xgs = self.xgp.tile([128, 16 * self.n * PF], BF16,
                            tag=f"xgs_{self.tag}")
        xv = xgs[:, :].rearrange("p (g b t) -> p g b t", g=16, t=PF)
        self.nc.sync.dma_start(xv[:, :, :, :pf], self.xg_ap[:, :, :, t0:t0 + pf])
        self.pending.append(xgs)

    def step(self, t):
        nc, n, gw = self.nc, self.n, self.gw
        if t % PF == 0:
            self.xgs = self.pending.pop(0)
            self.prefetch(t + PF)
        toff = t % PF
        xv = self.xgs[:, :].rearrange("p (g b t) -> p g b t", g=16, t=PF)

        # PE bank order [g, i, f, o]; type indices: i=0, f=1, o=2, g=3.
        # All four xg ident-matmuls first: they don't depend on h, so they
        # fill the PE while the previous step's tail (sig_o/mul_h) finishes.
        banks = {}
        for typ in (3, 0, 1, 2):
            ps = self.psp.tile([128, 512], F32, tag=f"ps{typ}_{self.tag}")
            banks[typ] = ps
            pv = ps[:, 0:gw].rearrange("p (g b) -> p g b", g=4)
            nc.tensor.matmul(
                pv[:, :, :], self.ident[:, :],
                xv[:, typ * 4:(typ + 1) * 4, :, toff],
                start=True, stop=(t == 0))
        for typ in (3, 0, 1, 2):
            ps = banks[typ]
            if t > 0:
                hcol = self.col_of_t(t - 1)
                for gcl in range(4):
                    gg = typ * 4 + gcl
                    for kc in range(4):
                        nc.tensor.matmul(
                            ps[:, gcl * n:(gcl + 1) * n],
                            self.whh[:, (kc * 16 + gg) * 128:
                                     (kc * 16 + gg + 1) * 128],
                            self.ob[:, kc, :, hcol],
                            start=False, stop=(kc == 3))
            # ALG, interleaved so each bank is consumed while PE moves on
            if typ == 3:      # tanh(g) -> st[0:gw]
                nc.scalar.activation(self.st[:, 0:gw], ps[:, 0:gw], AF.Tanh)
                self.acts = self.actp.tile([128, 3 * gw], F32,
                                           tag=f"acts_{self.tag}")
            elif typ == 0:    # sigmoid(i) -> acts[0:gw]
                nc.scalar.activation(self.acts[:, 0:gw], ps[:, 0:gw],
                                     AF.Sigmoid)
            elif typ == 1:    # sigmoid(f); then c-chain (independent of o)
                nc.scalar.activation(self.acts[:, gw:2 * gw], ps[:, 0:gw],
                                     AF.Sigmoid)
                prod = self.prodp.tile([128, 2 * gw], F32,
                                       tag=f"prod_{self.tag}")
                nc.vector.tensor_mul(prod[:, :], self.acts[:, 0:2 * gw],
                                     self.st[:, :])
                nc.vector.tensor_add(self.st[:, gw:2 * gw], prod[:, 0:gw],
                                     prod[:, gw:2 * gw])
                tc_ = self.tcp.tile([128, gw], F32, tag=f"tc_{self.tag}")
                nc.scalar.activation(tc_[:, :], self.st[:, gw:2 * gw], AF.Tanh)
                self.tc_ = tc_
            else:             # sigmoid(o); h = sig(o) * tanh(c) -> ob[..., t]
                nc.scalar.activation(self.acts[:, 2 * gw:3 * gw], ps[:, 0:gw],
                                     AF.Sigmoid)
                nc.vector.tensor_mul(
                    self.ob[:, :, :, self.col_of_t(t)],
                    self.acts[:, 2 * gw:3 * gw].rearrange(
                        "p (kc b) -> p kc b", kc=4),
                    self.tc_[:, :].rearrange("p (kc b) -> p kc b", kc=4))


def _lstm_phase(chains, T_steps):
    for ch in chains:
        ch.prefetch(0)
    for t in range(T_steps):
        for ch in chains:
            ch.step(t)


def _body(tc, nc, d, out, xg_f, xg_b, xg_d, S, T, n, NS, NT, TN, stages=5):
    n_tn = NT // TN

    with tc.tile_pool(name="persist", bufs=1) as pp:
        te = pp.tile([128, 4 * NS], BF16, tag="te")
        ctx = pp.tile([128, 4 * NT], BF16, tag="ctx")
        ident = _load(nc, pp, d, "ident", (128, 128), BF16)

        # ---------------- phase E0: encoder gate precompute ----------------
        with tc.tile_pool(name="e0w", bufs=1) as wp, \
             tc.tile_pool(name="e0ps", bufs=4, space="PSUM") as psp, \
             tc.tile_pool(name="e0sb", bufs=2) as sbp:
            tab_f = _load(nc, wp, d, "tab_f", (128, 4096), BF16)
            tab_b = _load(nc, wp, d, "tab_b", (128, 4096), BF16)
            oh_f = _load(nc, wp, d, "oh_f", (128, 2 * NS), BF16)
            oh_b = _load(nc, wp, d, "oh_b", (128, 2 * NS), BF16)
            _xg_produce(tc, nc, psp, sbp, tab_f, oh_f, xg_f, S, n)
            _xg_produce(tc, nc, psp, sbp, tab_b, oh_b, xg_b, S, n)
        if stages <= 1:
            return

        # ---------------- phase E1: encoder recurrences (interleaved) ------
        with tc.tile_pool(name="e1w", bufs=1) as ewp, \
             tc.tile_pool(name="e1buf", bufs=1) as ebp:
            whh_f = _load(nc, ewp, d, "whh_f", (128, 8192), BF16)
            whh_b = _load(nc, ewp, d, "whh_b", (128, 8192), BF16)
            buf_f = ebp.tile([128, 4 * NS], BF16, tag="buf_f")
            buf_b = ebp.tile([128, 4 * NS], BF16, tag="buf_b")

            with tc.tile_pool(name="e1ps", bufs=1, space="PSUM") as pls, \
                 tc.tile_pool(name="e1st", bufs=1) as stp, \
                 tc.tile_pool(name="e1act", bufs=2) as actp, \
                 tc.tile_pool(name="e1prod", bufs=2) as prodp, \
                 tc.tile_pool(name="e1tc", bufs=2) as tcp, \
                 tc.tile_pool(name="e1xg", bufs=3) as xgp:
                lp = (pls, stp, actp, prodp, tcp, xgp)
                chf = _LstmChain(nc, lp, S, n, xg_f, whh_f, buf_f[:, :],
                                 lambda t: t, ident, "f")
                chb = _LstmChain(nc, lp, S, n, xg_b, whh_b, buf_b[:, :],
                                 lambda t: S - 1 - t, ident, "b")
                _lstm_phase([chf, chb], S)

            # ---------------- phase E2: text projection -------------------
            with tc.tile_pool(name="e2w", bufs=1) as wtp, \
                 tc.tile_pool(name="e2ps", bufs=4, space="PSUM") as ptp:
                wt_f = _load(nc, wtp, d, "wt_f", (128, 2048), BF16)
                wt_b = _load(nc, wtp, d, "wt_b", (128, 2048), BF16)
                tb = _load(nc, wtp, d, "tb", (128, 4), F32)
                for mc in range(4):
                    for b in range(n):
                        ps = ptp.tile([128, 512], F32, tag="ps")
                        for kc in range(4):
                            nc.tensor.matmul(
                                ps[:, :S],
                                wt_f[:, (kc * 4 + mc) * 128:(kc * 4 + mc + 1) * 128],
                                buf_f[:, kc * NS + b * S:kc * NS + (b + 1) * S],
                                start=(kc == 0), stop=False)
                        for kc in range(4):
                            nc.tensor.matmul(
                                ps[:, :S],
                                wt_b[:, (kc * 4 + mc) * 128:(kc * 4 + mc + 1) * 128],
                                buf_b[:, kc * NS + b * S:kc * NS + (b + 1) * S],
                                start=False, stop=(kc == 3))
                        dst = te[:, mc * NS + b * S:mc * NS + (b + 1) * S]
                        if (mc * n + b) % 2 == 0:
                            nc.scalar.activation(dst, ps[:, :S], AF.Identity,
                                                 bias=tb[:, mc:mc + 1])
                        else:
                            nc.vector.tensor_scalar_add(dst, ps[:, :S],
                                                        tb[:, mc:mc + 1])

        if stages <= 2:
            return
        # ------- phase A+X: attention, interleaved with decoder gates -----
        melT = pp.tile([81, NT], BF16, tag="melT")
        nc.sync.dma_start(melT[:, :], d["melT"].ap())
        _attention(tc, nc, d, te, ctx, melT, S, T, n, NS, NT, TN, xg_d,
                   stages)

        if stages <= 4:
            return
        # ---------------- phase D: decoder recurrence + out proj ----------
        with tc.tile_pool(name="dw", bufs=1) as dwp, \
             tc.tile_pool(name="dbuf", bufs=1) as dbp:
            whh_d = _load(nc, dwp, d, "whh_d", (128, 8192), BF16)
            dbuf = dbp.tile([128, 4 * NT], BF16, tag="dbuf")

            with tc.tile_pool(name="dps", bufs=2, space="PSUM") as pls, \
                 tc.tile_pool(name="dst", bufs=1) as stp, \
                 tc.tile_pool(name="dact", bufs=2) as actp, \
                 tc.tile_pool(name="dprod", bufs=2) as prodp, \
                 tc.tile_pool(name="dtc", bufs=2) as tcp, \
                 tc.tile_pool(name="dxg", bufs=3) as xgp:
                lp = (pls, stp, actp, prodp, tcp, xgp)
                chd = _LstmChain(nc, lp, T, n, xg_d, whh_d, dbuf[:, :],
                                 lambda t: t, ident, "d")
                _lstm_phase([chd], T)

            with tc.tile_pool(name="ow", bufs=1) as mop, \
                 tc.tile_pool(name="ops", bufs=4, space="PSUM") as pso, \
                 tc.tile_pool(name="osb", bufs=4) as sbo:
                mo = _load(nc, mop, d, "mo", (128, 320), BF16)
                mob = _load(nc, mop, d, "mob", (80, 1), F32)
                for nt in range(n_tn):
                    ps = pso.tile([80, TN], F32, tag="ps")
                    for kc in range(4):
                        nc.tensor.matmul(
                            ps[:, :], mo[:, kc * 80:(kc + 1) * 80],
                            dbuf[:, kc * NT + nt * TN:kc * NT + (nt + 1) * TN],
                            start=(kc == 0), stop=(kc == 3))
                    sb = sbo.tile([80, TN], F32, tag="out_sb")
                    nc.scalar.activation(sb[:, :], ps[:, :], AF.Identity,
                                         bias=mob[:, :])
                    nc.sync.dma_start(out.ap()[:, nt * TN:(nt + 1) * TN],
                                      sb[:, :])


def _attention(tc, nc, d, te, ctx, melT, S, T, n, NS, NT, TN, xg_d, stages):
    n_tn = NT // TN
    n_sc = NS // 128
    with tc.tile_pool(name="aw", bufs=1) as awp, \
         tc.tile_pool(name="aps", bufs=4, space="PSUM") as pa, \
         tc.tile_pool(name="aqt", bufs=1) as qtp, \
         tc.tile_pool(name="akt", bufs=1) as ktp, \
         tc.tile_pool(name="avs", bufs=1) as vsp, \
         tc.tile_pool(name="aet", bufs=2) as etp, \
         tc.tile_pool(name="actx", bufs=3) as cxp, \
         tc.tile_pool(name="xsb", bufs=1) as dsb:

        wk = _load(nc, awp, d, "wk", (128, 2048), BF16)
        wv = _load(nc, awp, d, "wv", (128, 2048), BF16)
        vb = _load(nc, awp, d, "vb", (64, 8), F32)
        qw = _load(nc, awp, d, "qw", (81, 512), BF16)
        do_xd = stages > 3
        if do_xd:
            m1 = _load(nc, awp, d, "m1", (81, 2048), BF16)
            m2 = _load(nc, awp, d, "m2", (128, 4 * 2048), BF16)
            xd_ap = xg_d.ap().rearrange("p (g b t) -> p g b t", g=16, b=n)
        ones = awp.tile([1, 64], F32, tag="ones64")
        nc.vector.memset(ones[:, :], 1.0)

        # QT sbuf-resident: [128, 4mc x NT]
        qt = qtp.tile([128, 4 * NT], BF16, tag="qt")
        for mc in range(4):
            for nt in range(n_tn):
                ps = pa.tile([128, TN], F32, tag="ps")
                nc.tensor.matmul(ps[:, :], qw[:, mc * 128:(mc + 1) * 128],
                                 melT[:, nt * TN:(nt + 1) * TN],
                                 start=True, stop=True)
                _plain_copy(nc, mc * n_tn + nt,
                            qt[:, mc * NT + nt * TN:mc * NT + (nt + 1) * TN],
                            ps[:, :])

        # KT sbuf-resident: [128, 4mc x NS]
        kt = ktp.tile([128, 4 * NS], BF16, tag="kt")
        for mc in range(4):
            for b in range(n):
                ps = pa.tile([128, 512], F32, tag="ps")
                for kc in range(4):
                    nc.tensor.matmul(
                        ps[:, :S],
                        wk[:, (kc * 4 + mc) * 128:(kc * 4 + mc + 1) * 128],
                        te[:, kc * NS + b * S:kc * NS + (b + 1) * S],
                        start=(kc == 0), stop=(kc == 3))
                _plain_copy(nc, mc * n + b,
                            kt[:, mc * NS + b * S:mc * NS + (b + 1) * S],
                            ps[:, :S])

        # V with ones column per head: [128(s-sub), n_sc x (8h x 65)]
        vsb = vsp.tile([128, n_sc * 520], BF16, tag="vsb")
        for sc in range(n_sc):
            ps = pa.tile([128, 512], F32, tag="ps")
            for kc in range(4):
                nc.tensor.matmul(
                    ps[:, :], te[:, kc * NS + sc * 128:kc * NS + sc * 128 + 128],
                    wv[:, kc * 512:(kc + 1) * 512],
                    start=(kc == 0), stop=(kc == 3))
            dst = vsb[:, sc * 520:(sc + 1) * 520].rearrange("p (h c) -> p h c", h=8)
            _plain_copy(nc, sc, dst[:, :, 0:64],
                        ps[:, :].rearrange("p (h c) -> p h c", h=8))
            nc.vector.memset(dst[:, :, 64:65], 1.0)

        def xd_produce_b(b):
            """Decoder gate precompute for example b (PE-heavy; overlaps the
            next example's attention, which is ACT-heavy)."""
            for tbk in range(T // TN):
                st = dsb.tile([128, 16 * TN], BF16, tag="xd_stage")
                sv = st[:, :].rearrange("p (g t) -> p g t", g=16)
                for gc in range(16):
                    ps = pa.tile([128, 512], F32, tag="ps")
                    nc.tensor.matmul(
                        ps[:, :TN], m1[:, gc * 128:(gc + 1) * 128],
                        melT[:, b * T + tbk * TN:b * T + (tbk + 1) * TN],
                        start=True, stop=False)
                    for hc in range(4):
                        nc.tensor.matmul(
                            ps[:, :TN],
                            m2[:, (hc * 16 + gc) * 128:(hc * 16 + gc + 1) * 128],
                            ctx[:, hc * NT + b * T + tbk * TN:
                                hc * NT + b * T + (tbk + 1) * TN],
                            start=False, stop=(hc == 3))
                    _plain_copy(nc, b * 16 + gc, sv[:, gc, :], ps[:, :TN])
                nc.sync.dma_start(xd_ap[:, :, b, tbk * TN:(tbk + 1) * TN],
                                  sv[:, :, :])

        # per (example, head): scoresT -> exp -> ctx + sums -> scale
        nsc_b = S // 128  # s-chunks per example
        for b in range(n):
            if do_xd and b > 0:
                xd_produce_b(b - 1)
            for h in range(NH):
                hc, hr = h // 2, (h % 2) * 64
                et = etp.tile([128, nsc_b * T], BF16, tag="et")
                for tt in range(T // TN):
                    qs = qt[hr:hr + 64,
                            hc * NT + b * T + tt * TN:hc * NT + b * T + (tt + 1) * TN]
                    for scl in range(nsc_b):
                        ps = pa.tile([128, TN], F32, tag="ps")
                        nc.tensor.matmul(
                            ps[:, :],
                            kt[hr:hr + 64,
                               hc * NS + b * S + scl * 128:hc * NS + b * S + scl * 128 + 128],
                            qs, start=True, stop=True)
                        nc.scalar.activation(
                            et[:, scl * T + tt * TN:scl * T + (tt + 1) * TN],
                            ps[:, :], AF.Exp)
                cps = [pa.tile([65, TN], F32, tag="ps", name=f"cps{tt}")
                       for tt in range(T // TN)]
                for scl in range(nsc_b):
                    lhs = vsb[:, (b * nsc_b + scl) * 520 + h * 65:
                              (b * nsc_b + scl) * 520 + (h + 1) * 65]
                    for tt in range(T // TN):
                        nc.tensor.matmul(cps[tt][:, :], lhs,
                                         et[:, scl * T + tt * TN:scl * T + (tt + 1) * TN],
                                         start=(scl == 0), stop=(scl == nsc_b - 1))
                for tt in range(T // TN):
                    rc = cxp.tile([1, TN], F32, tag="recip")
                    nc.vector.reciprocal(rc[:, :], cps[tt][64:65, :])
                    rb = pa.tile([64, TN], F32, tag="ps")
                    nc.tensor.matmul(rb[:, :], ones[:, :], rc[:, :],
                                     start=True, stop=True)
                    c0 = cxp.tile([64, TN], F32, tag="ctx_unsc")
                    _plain_copy(nc, b * NH + h + tt, c0[:, :], cps[tt][0:64, :])
                    sc1 = cxp.tile([64, TN], F32, tag="ctx_scaled")
                    nc.vector.tensor_mul(sc1[:, :], c0[:, :], rb[:, :])
                    nc.vector.tensor_scalar_add(
                        ctx[hr:hr + 64,
                            hc * NT + b * T + tt * TN:hc * NT + b * T + (tt + 1) * TN],
                        sc1[:, :], vb[:, h:h + 1])
        if do_xd:
            xd_produce_b(n - 1)


# ---------------------------------------------------------------------------
# host wrapper
# ---------------------------------------------------------------------------

def prep_host(inputs, S, T, n_per_core, ncores):
    f32 = np.float32
    emb = np.asarray(inputs["emb"], f32)
    idx = np.asarray(inputs["phoneme_indices"]).astype(np.int64)
    mel = np.asarray(inputs["mel_specs"], f32)

    bias_f = np.asarray(inputs["enc_bih_f"], f32) + np.asarray(inputs["enc_bhh_f"], f32)
    bias_b = np.asarray(inputs["enc_bih_b"], f32) + np.asarray(inputs["enc_bhh_b"], f32)
    tab_f = (emb @ np.asarray(inputs["enc_Wih_f"], f32).T + bias_f)[:, GPERM]
    tab_b = (emb @ np.asarray(inputs["enc_Wih_b"], f32).T + bias_b)[:, GPERM]

    tproj_W = np.asarray(inputs["tproj_W"], f32)
    tproj_b = np.asarray(inputs["tproj_b"], f32)
    Wq, Wk, Wv = np.split(np.asarray(inputs["attn_in_W"], f32), 3, axis=0)
    bq, bk, bv = np.split(np.asarray(inputs["attn_in_b"], f32), 3)
    mpw = np.asarray(inputs["mproj_in_W"], f32)
    mpb = np.asarray(inputs["mproj_in_b"], f32)
    aow = np.asarray(inputs["attn_out_W"], f32)
    aob = np.asarray(inputs["attn_out_b"], f32)
    dWih = np.asarray(inputs["dec_Wih"], f32)[GPERM]
    dbias = (np.asarray(inputs["dec_bih"], f32)
             + np.asarray(inputs["dec_bhh"], f32))[GPERM]
    mow = np.asarray(inputs["mproj_out_W"], f32)
    mob = np.asarray(inputs["mproj_out_b"], f32)

    scale = f32(1.0) / np.sqrt(f32(HD))
    QW = (Wq @ mpw) * scale
    qb_ = (bq + Wq @ mpb) * scale
    vb_ = bv                       # tproj_b folded into te; bk softmax-invariant
    W1, W2 = dWih[:, :H], dWih[:, H:]
    M1 = W1 @ mpw
    M2 = W2 @ aow
    dbias_ = dbias + W1 @ mpb + W2 @ aob

    whh_f_p = np.asarray(inputs["enc_Whh_f"], f32)[GPERM]
    whh_b_p = np.asarray(inputs["enc_Whh_b"], f32)[GPERM]
    whh_d_p = np.asarray(inputs["dec_Whh"], f32)[GPERM]

    def bf(a):
        return np.ascontiguousarray(a.astype(BF))

    common = {
        "ident": bf(np.eye(128, dtype=f32)),
        "tab_f": bf(_lhsT_tiles(tab_f)),
        "tab_b": bf(_lhsT_tiles(tab_b)),
        "whh_f": bf(_lhsT_tiles(whh_f_p.T)),
        "whh_b": bf(_lhsT_tiles(whh_b_p.T)),
        "whh_d": bf(_lhsT_tiles(whh_d_p.T)),
        "wt_f": bf(_lhsT_tiles(tproj_W[:, :H].T)),
        "wt_b": bf(_lhsT_tiles(tproj_W[:, H:].T)),
        "tb": np.ascontiguousarray(tproj_b.reshape(4, 128).T.astype(f32)),
        "wk": bf(_lhsT_tiles(Wk.T)),
        "wv": bf(Wv.T.reshape(4, 128, 512).transpose(1, 0, 2).reshape(128, 2048)),
        "vb": np.ascontiguousarray(vb_.reshape(8, 64).T.astype(f32)),
        "qw": bf(np.concatenate([QW.T, qb_.reshape(1, 512)], axis=0)),
        "m1": bf(np.concatenate([M1.T, dbias_.reshape(1, 2048)], axis=0)),
        "m2": bf(M2.T.reshape(4, 128, 2048).transpose(1, 0, 2).reshape(128, 4 * 2048)),
        "mo": bf(mow.T.reshape(4, 128, 80).transpose(1, 0, 2).reshape(128, 320)),
        "mob": np.ascontiguousarray(mob.reshape(80, 1).astype(f32)),
    }

    shifted = np.concatenate([np.zeros_like(mel[:, :1]), mel[:, :-1]], axis=1)

    in_maps = []
    for c in range(ncores):
        exs = list(range(c * n_per_core, (c + 1) * n_per_core))
        ohf = np.zeros((VOCAB, n_per_core * S), f32)
        ohb = np.zeros((VOCAB, n_per_core * S), f32)
        cols = np.arange(S)
        for bi, e in enumerate(exs):
            ohf[idx[e, :S], bi * S + cols] = 1.0
            ohb[idx[e, S - 1 - cols], bi * S + cols] = 1.0
        melTc = np.ones((MEL + 1, n_per_core * T), f32)
        for bi, e in enumerate(exs):
            melTc[:MEL, bi * T:(bi + 1) * T] = shifted[e, :T].T
        m = dict(common)
        m["oh_f"] = bf(np.concatenate([ohf[:128], ohf[128:]], axis=1))
        m["oh_b"] = bf(np.concatenate([ohb[:128], ohb[128:]], axis=1))
        m["melT"] = bf(melTc)
        in_maps.append(m)
    return in_maps


def run(inputs, S, T, n, ncores, trace=False):
    nc = build_program(S=S, T=T, n=n)
    in_maps = prep_host(inputs, S, T, n, ncores)
    res = bass_utils.run_bass_kernel_spmd(
        nc, in_maps, core_ids=list(range(ncores)), trace=trace)
    Bt = n * ncores
    out = np.zeros((Bt, T, MEL), np.float32)
    for c in range(ncores):
        o = np.asarray(res.results[c]["out"])
        for bi in range(n):
            out[c * n + bi] = o[:, bi * T:(bi + 1) * T].T
    return out, res


def kernel(**inputs):
    out, _ = run(inputs, S_FULL, T_FULL, B_FULL // NCORES, NCORES)
    return out


# revision 9
# speedup vs baseline: 12.9774x; 1.2218x over previous
# Trainium2 Bass kernel for nn_KokoroModel (text->mel seq2seq, LSTM enc/dec + MHA).
#
# Sharding: data-parallel over batch, 4 examples/core on 8 cores, weights
# replicated. Host-side weight-only fusions (model reparametrizations):
#   TAB  = emb @ enc_Wih.T            (embedding gather -> one-hot matmul)
#   QW   = (Wq @ mproj_in_W) / 8      (mel proj + Q proj + attn scale)
#   M1   = dec_Wih[:, :H] @ mproj_in_W
#   M2   = dec_Wih[:, H:] @ attn_out_W
#   K/V biases absorb tproj_b; decoder gate bias absorbs mproj_in/attn_out biases.
# Softmax normalization is deferred: exp-scores stay unnormalized, row-sums come
# from a ones-column appended to V, context is scaled by 1/sum afterwards.
#
# LSTM recurrence design (the serial bottleneck, 2024 steps):
#  - Gate types go to four separate PSUM banks so ACT can read a finished
#    bank while PE writes the next (PSUM collisions are bank-level). PE
#    emission order is [g, i, f, o]: the c-update chain (needs g,i,f) hides
#    under the o-bank matmuls; only sigmoid(o) + final h-mul trail the stream.
#  - xg (input gates) are precomputed to DRAM laid out [128,(type,gc,b,t)]
#    (t contiguous) and streamed in PF-step slabs, prefetched one slab ahead;
#    an identity-matmul accumulates xg into each PSUM bank (no DVE add).
#  - h_t is written by the final DVE mul directly (strided) into the SBUF
#    h-history buffer in (kc, b, t) layout; next-step matmuls read strided
#    columns from there. No per-step DMA.
#  - Encoder fwd/bwd chains interleave per tick: one direction's gate algebra
#    overlaps the other's matmul stream.

import numpy as np
import ml_dtypes

import concourse.bass as bass
import concourse.tile as tile
from concourse import bacc, mybir
from concourse import bass_utils

F32 = mybir.dt.float32
BF16 = mybir.dt.bfloat16
AF = mybir.ActivationFunctionType
BF = ml_dtypes.bfloat16

VOCAB, MEL, H = 256, 80, 512
NH, HD = 8, 64
B_FULL, S_FULL, T_FULL = 32, 512, 1000
NCORES = 8
PF = 16  # xg slab length (steps per prefetch)

# gate-type permutation: torch order [i,f,g,o] -> ours [i,f,o,g] (type-major)
GPERM = np.r_[0:H, H:2 * H, 3 * H:4 * H, 2 * H:3 * H]


def _lhsT_tiles(w, kp=128, mp=128):
    """w: [K, M] -> [kp, (K//kp)*(M//mp)*mp]; tile (kc,mc) at cols
    (kc*(M//mp)+mc)*mp."""
    K, M = w.shape
    nk, nm = K // kp, M // mp
    return np.ascontiguousarray(
        w.reshape(nk, kp, nm, mp).transpose(1, 0, 2, 3).reshape(kp, nk * nm * mp))


# ---------------------------------------------------------------------------
# device program
# ---------------------------------------------------------------------------

def build_program(S=S_FULL, T=T_FULL, n=B_FULL // NCORES, stages=5):
    nc = bacc.Bacc("TRN2", target_bir_lowering=False, debug=False)

    NS, NT = n * S, n * T
    TN = T // 2       # mel matmul N-tile
    assert TN <= 512

    d = {}

    def din(name, shape, dt):
        d[name] = nc.dram_tensor(name, list(shape), dt, kind="ExternalInput")

    din("ident", (128, 128), BF16)
    din("oh_f", (128, 2 * NS), BF16)
    din("oh_b", (128, 2 * NS), BF16)
    din("tab_f", (128, 2 * 16 * 128), BF16)   # stationary tiles (vc, gc)
    din("tab_b", (128, 2 * 16 * 128), BF16)
    din("whh_f", (128, 4 * 16 * 128), BF16)   # stationary tiles (kc, gc)
    din("whh_b", (128, 4 * 16 * 128), BF16)
    din("whh_d", (128, 4 * 16 * 128), BF16)
    din("wt_f", (128, 4 * 4 * 128), BF16)
    din("wt_b", (128, 4 * 4 * 128), BF16)
    din("tb", (128, 4), F32)
    din("wk", (128, 4 * 4 * 128), BF16)
    din("wv", (128, 4 * 512), BF16)
    din("vb", (64, 8), F32)
    din("qw", (81, 4 * 128), BF16)
    din("melT", (81, NT), BF16)
    din("m1", (81, 16 * 128), BF16)           # stationary tiles (gc)
    din("m2", (128, 4 * 16 * 128), BF16)      # stationary tiles (hc, gc)
    din("mo", (128, 4 * 80), BF16)
    din("mob", (80, 1), F32)

    out = nc.dram_tensor("out", [80, NT], F32, kind="ExternalOutput")

    # xg layout: [128, (type4*gc4, b, t)] -- t contiguous per (type,gc,b)
    xg_f = nc.dram_tensor("xg_f", [128, 16 * NS], BF16, kind="Internal")
    xg_b = nc.dram_tensor("xg_b", [128, 16 * NS], BF16, kind="Internal")
    xg_d = nc.dram_tensor("xg_d", [128, 16 * NT], BF16, kind="Internal")

    with tile.TileContext(nc) as tc:
        _body(tc, nc, d, out, xg_f, xg_b, xg_d, S, T, n, NS, NT, TN, stages)

    nc.compile()
    return nc


def _plain_copy(nc, i, dst, src):
    if i % 2 == 0:
        nc.scalar.copy(dst, src)
    else:
        nc.vector.tensor_copy(dst, src)


def _load(nc, pool, d, name, shape, dt):
    t_ = pool.tile(list(shape), dt, tag=name)
    nc.sync.dma_start(t_[:, :], d[name].ap())
    return t_


def _xg_produce(tc, nc, psp, sbp, tab, oh, xg_dram, S, n):
    """xg[p, (g, b, t)] = (TAB_tile.T @ onehot)[gate, token]; bias folded in.
    Gates land on partitions; DRAM runs are contiguous in t."""
    NS = n * S
    TB = min(S, 512)
    nsb = S // TB
    xg_ap = xg_dram.ap().rearrange("p (g b t) -> p g b t", g=16, b=n)
    i = 0
    for b in range(n):
        st = sbp.tile([128, 16 * S], BF16, tag="xg_stage")
        sv = st[:, :].rearrange("p (g t) -> p g t", g=16)
        for gc in range(16):
            for tb in range(nsb):
                ps = psp.tile([128, 512], F32, tag="ps")
                col0 = b * S + tb * TB
                for vc in range(2):
                    nc.tensor.matmul(
                        ps[:, :TB],
                        tab[:, (vc * 16 + gc) * 128:(vc * 16 + gc + 1) * 128],
                        oh[:, vc * NS + col0:vc * NS + col0 + TB],
                        start=(vc == 0), stop=(vc == 1))
                _plain_copy(nc, i, sv[:, gc, tb * TB:(tb + 1) * TB], ps[:, :TB])
                i += 1
        nc.sync.dma_start(xg_ap[:, :, b, :], sv[:, :, :])


class _LstmChain:
    """State + per-step emission for one LSTM recurrence."""

    def __init__(self, nc, pools, T_steps, n, xg_dram, whh, ob, col_of_t,
                 ident, tag):
        self.nc = nc
        (self.psp, self.stp, self.actp, self.prodp, self.tcp, self.xgp) = pools
        self.T_steps, self.n = T_steps, n
        self.xg_ap = xg_dram.ap().rearrange("p (g b t) -> p g b t", g=16, b=n)
        self.whh = whh
        self.ob = ob.rearrange("p (kc b t) -> p kc b t", kc=4, b=n)
        self.col_of_t = col_of_t
        self.ident = ident
        self.tag = tag
        self.gw = 4 * n
        self.pending = []
        st = self.stp.tile([128, 2 * self.gw], F32, tag=f"st_{tag}")
        nc.vector.memset(st[:, self.gw:2 * self.gw], 0.0)   # c = 0
        self.st = st
        self.xgs = None

    def prefetch(self, t0):
        if t0 >= self.T_steps:
            return
        pf = min(PF, self.T_steps - t0)
        xgs = self.xgp.tile([128, 16 * self.n * PF], BF16,
                            tag=f"xgs_{self.tag}")
        xv = xgs[:, :].rearrange("p (g b t) -> p g b t", g=16, t=PF)
        self.nc.sync.dma_start(xv[:, :, :, :pf], self.xg_ap[:, :, :, t0:t0 + pf])
        self.pending.append(xgs)

    def step(self, t):
        nc, n, gw = self.nc, self.n, self.gw
        if t % PF == 0:
            self.xgs = self.pending.pop(0)
            self.prefetch(t + PF)
        toff = t % PF
        xv = self.xgs[:, :].rearrange("p (g b t) -> p g b t", g=16, t=PF)

        # PE bank order [g, i, f, o]; type indices: i=0, f=1, o=2, g=3.
        # All four xg ident-matmuls first: they don't depend on h, so they
        # fill the PE while the previous step's tail (sig_o/mul_h) finishes.
        banks = {}
        for typ in (3, 0, 1, 2):
            ps = self.psp.tile([128, 512], F32, tag=f"ps{typ}_{self.tag}")
            banks[typ] = ps
            pv = ps[:, 0:gw].rearrange("p (g b) -> p g b", g=4)
            nc.tensor.matmul(
                pv[:, :, :], self.ident[:, :],
                xv[:, typ * 4:(typ + 1) * 4, :, toff],
                start=True, stop=(t == 0))
        for typ in (3, 0, 1, 2):
            ps = banks[typ]
            if t > 0:
                hcol = self.col_of_t(t - 1)
                for gcl in range(4):
                    gg = typ * 4 + gcl
                    for kc in range(4):
                        nc.tensor.matmul(
                            ps[:, gcl * n:(gcl + 1) * n],
                            self.whh[:, (kc * 16 + gg) * 128:
                                     (kc * 16 + gg + 1) * 128],
                            self.ob[:, kc, :, hcol],
                            start=False, stop=(kc == 3))
            # ALG, interleaved so each bank is consumed while PE moves on
            if typ == 3:      # tanh(g) -> st[0:gw]
                nc.scalar.activation(self.st[:, 0:gw], ps[:, 0:gw], AF.Tanh)
                self.acts = self.actp.tile([128, 3 * gw], F32,
                                           tag=f"acts_{self.tag}")
            elif typ == 0:    # sigmoid(i) -> acts[0:gw]
                nc.scalar.activation(self.acts[:, 0:gw], ps[:, 0:gw],
                                     AF.Sigmoid)
            elif typ == 1:    # sigmoid(f); then c-chain (independent of o)
                nc.scalar.activation(self.acts[:, gw:2 * gw], ps[:, 0:gw],
                                     AF.Sigmoid)
                prod = self.prodp.tile([128, 2 * gw], F32,
                                       tag=f"prod_{self.tag}")
                nc.vector.tensor_mul(prod[:, :], self.acts[:, 0:2 * gw],
                                     self.st[:, :])
                nc.vector.tensor_add(self.st[:, gw:2 * gw], prod[:, 0:gw],
                                     prod[:, gw:2 * gw])
                tc_ = self.tcp.tile([128, gw], F32, tag=f"tc_{self.tag}")
                nc.scalar.activation(tc_[:, :], self.st[:, gw:2 * gw], AF.Tanh)
                self.tc_ = tc_
            else:             # sigmoid(o); h = sig(o) * tanh(c) -> ob[..., t]
                nc.scalar.activation(self.acts[:, 2 * gw:3 * gw], ps[:, 0:gw],
                                     AF.Sigmoid)
                nc.vector.tensor_mul(
                    self.ob[:, :, :, self.col_of_t(t)],
                    self.acts[:, 2 * gw:3 * gw].rearrange(
                        "p (kc b) -> p kc b", kc=4),
                    self.tc_[:, :].rearrange("p (kc b) -> p kc b", kc=4))


def _lstm_phase(chains, T_steps):
    for ch in chains:
        ch.prefetch(0)
    for t in range(T_steps):
        for ch in chains:
            ch.step(t)


def _body(tc, nc, d, out, xg_f, xg_b, xg_d, S, T, n, NS, NT, TN, stages=5):
    n_tn = NT // TN

    with tc.tile_pool(name="persist", bufs=1) as pp:
        te = pp.tile([128, 4 * NS], BF16, tag="te")
        ctx = pp.tile([128, 4 * NT], BF16, tag="ctx")
        ident = _load(nc, pp, d, "ident", (128, 128), BF16)

        # ---------------- phase E0: encoder gate precompute ----------------
        with tc.tile_pool(name="e0w", bufs=1) as wp, \
             tc.tile_pool(name="e0ps", bufs=4, space="PSUM") as psp, \
             tc.tile_pool(name="e0sb", bufs=2) as sbp:
            tab_f = _load(nc, wp, d, "tab_f", (128, 4096), BF16)
            tab_b = _load(nc, wp, d, "tab_b", (128, 4096), BF16)
            oh_f = _load(nc, wp, d, "oh_f", (128, 2 * NS), BF16)
            oh_b = _load(nc, wp, d, "oh_b", (128, 2 * NS), BF16)
            _xg_produce(tc, nc, psp, sbp, tab_f, oh_f, xg_f, S, n)
            _xg_produce(tc, nc, psp, sbp, tab_b, oh_b, xg_b, S, n)
        if stages <= 1:
            return

        # ---------------- phase E1: encoder recurrences (interleaved) ------
        with tc.tile_pool(name="e1w", bufs=1) as ewp, \
             tc.tile_pool(name="e1buf", bufs=1) as ebp:
            whh_f = _load(nc, ewp, d, "whh_f", (128, 8192), BF16)
            whh_b = _load(nc, ewp, d, "whh_b", (128, 8192), BF16)
            buf_f = ebp.tile([128, 4 * NS], BF16, tag="buf_f")
            buf_b = ebp.tile([128, 4 * NS], BF16, tag="buf_b")

            with tc.tile_pool(name="e1ps", bufs=1, space="PSUM") as pls, \
                 tc.tile_pool(name="e1st", bufs=1) as stp, \
                 tc.tile_pool(name="e1act", bufs=2) as actp, \
                 tc.tile_pool(name="e1prod", bufs=2) as prodp, \
                 tc.tile_pool(name="e1tc", bufs=2) as tcp, \
                 tc.tile_pool(name="e1xg", bufs=3) as xgp:
                lp = (pls, stp, actp, prodp, tcp, xgp)
                chf = _LstmChain(nc, lp, S, n, xg_f, whh_f, buf_f[:, :],
                                 lambda t: t, ident, "f")
                chb = _LstmChain(nc, lp, S, n, xg_b, whh_b, buf_b[:, :],
                                 lambda t: S - 1 - t, ident, "b")
                _lstm_phase([chf, chb], S)

            # ---------------- phase E2: text projection -------------------
            with tc.tile_pool(name="e2w", bufs=1) as wtp, \
                 tc.tile_pool(name="e2ps", bufs=4, space="PSUM") as ptp:
                wt_f = _load(nc, wtp, d, "wt_f", (128, 2048), BF16)
                wt_b = _load(nc, wtp, d, "wt_b", (128, 2048), BF16)
                tb = _load(nc, wtp, d, "tb", (128, 4), F32)
                for mc in range(4):
                    for b in range(n):
                        ps = ptp.tile([128, 512], F32, tag="ps")
                        for kc in range(4):
                            nc.tensor.matmul(
                                ps[:, :S],
                                wt_f[:, (kc * 4 + mc) * 128:(kc * 4 + mc + 1) * 128],
                                buf_f[:, kc * NS + b * S:kc * NS + (b + 1) * S],
                                start=(kc == 0), stop=False)
                        for kc in range(4):
                            nc.tensor.matmul(
                                ps[:, :S],
                                wt_b[:, (kc * 4 + mc) * 128:(kc * 4 + mc + 1) * 128],
                                buf_b[:, kc * NS + b * S:kc * NS + (b + 1) * S],
                                start=False, stop=(kc == 3))
                        dst = te[:, mc * NS + b * S:mc * NS + (b + 1) * S]
                        if (mc * n + b) % 2 == 0:
                            nc.scalar.activation(dst, ps[:, :S], AF.Identity,
                                                 bias=tb[:, mc:mc + 1])
                        else:
                            nc.vector.tensor_scalar_add(dst, ps[:, :S],
                                                        tb[:, mc:mc + 1])

        if stages <= 2:
            return
        # ------- phase A+X: attention, interleaved with decoder gates -----
        melT = pp.tile([81, NT], BF16, tag="melT")
        nc.sync.dma_start(melT[:, :], d["melT"].ap())
        _attention(tc, nc, d, te, ctx, melT, S, T, n, NS, NT, TN, xg_d,
                   stages)

        if stages <= 4:
            return
        # ---------------- phase D: decoder recurrence + out proj ----------
        with tc.tile_pool(name="dw", bufs=1) as dwp, \
             tc.tile_pool(name="dbuf", bufs=1) as dbp:
            whh_d = _load(nc, dwp, d, "whh_d", (128, 8192), BF16)
            dbuf = dbp.tile([128, 4 * NT], BF16, tag="dbuf")

            with tc.tile_pool(name="dps", bufs=2, space="PSUM") as pls, \
                 tc.tile_pool(name="dst", bufs=1) as stp, \
                 tc.tile_pool(name="dact", bufs=2) as actp, \
                 tc.tile_pool(name="dprod", bufs=2) as prodp, \
                 tc.tile_pool(name="dtc", bufs=2) as tcp, \
                 tc.tile_pool(name="dxg", bufs=3) as xgp:
                lp = (pls, stp, actp, prodp, tcp, xgp)
                chd = _LstmChain(nc, lp, T, n, xg_d, whh_d, dbuf[:, :],
                                 lambda t: t, ident, "d")
                _lstm_phase([chd], T)

            with tc.tile_pool(name="ow", bufs=1) as mop, \
                 tc.tile_pool(name="ops", bufs=4, space="PSUM") as pso, \
                 tc.tile_pool(name="osb", bufs=4) as sbo:
                mo = _load(nc, mop, d, "mo", (128, 320), BF16)
                mob = _load(nc, mop, d, "mob", (80, 1), F32)
                for nt in range(n_tn):
                    ps = pso.tile([80, TN], F32, tag="ps")
                    for kc in range(4):
                        nc.tensor.matmul(
                            ps[:, :], mo[:, kc * 80:(kc + 1) * 80],
                            dbuf[:, kc * NT + nt * TN:kc * NT + (nt + 1) * TN],
                            start=(kc == 0), stop=(kc == 3))
                    sb = sbo.tile([80, TN], F32, tag="out_sb")
                    nc.scalar.activation(sb[:, :], ps[:, :], AF.Identity,
                                         bias=mob[:, :])
                    nc.sync.dma_start(out.ap()[:, nt * TN:(nt + 1) * TN],
                                      sb[:, :])


def _attention(tc, nc, d, te, ctx, melT, S, T, n, NS, NT, TN, xg_d, stages):
    n_tn = NT // TN
    n_sc = NS // 128
    with tc.tile_pool(name="aw", bufs=1) as awp, \
         tc.tile_pool(name="aps", bufs=4, space="PSUM") as pa, \
         tc.tile_pool(name="aqt", bufs=1) as qtp, \
         tc.tile_pool(name="akt", bufs=1) as ktp, \
         tc.tile_pool(name="avs", bufs=1) as vsp, \
         tc.tile_pool(name="aet", bufs=2) as etp, \
         tc.tile_pool(name="actx", bufs=3) as cxp, \
         tc.tile_pool(name="xsb", bufs=1) as dsb:

        wk = _load(nc, awp, d, "wk", (128, 2048), BF16)
        wv = _load(nc, awp, d, "wv", (128, 2048), BF16)
        vb = _load(nc, awp, d, "vb", (64, 8), F32)
        qw = _load(nc, awp, d, "qw", (81, 512), BF16)
        do_xd = stages > 3
        if do_xd:
            m1 = _load(nc, awp, d, "m1", (81, 2048), BF16)
            m2 = _load(nc, awp, d, "m2", (128, 4 * 2048), BF16)
            xd_ap = xg_d.ap().rearrange("p (g b t) -> p g b t", g=16, b=n)
        ones = awp.tile([1, 64], F32, tag="ones64")
        nc.vector.memset(ones[:, :], 1.0)

        # QT sbuf-resident: [128, 4mc x NT]
        qt = qtp.tile([128, 4 * NT], BF16, tag="qt")
        for mc in range(4):
            for nt in range(n_tn):
                ps = pa.tile([128, TN], F32, tag="ps")
                nc.tensor.matmul(ps[:, :], qw[:, mc * 128:(mc + 1) * 128],
                                 melT[:, nt * TN:(nt + 1) * TN],
                                 start=True, stop=True)
                _plain_copy(nc, mc * n_tn + nt,
                            qt[:, mc * NT + nt * TN:mc * NT + (nt + 1) * TN],
                            ps[:, :])

        # KT sbuf-resident: [128, 4mc x NS]
        kt = ktp.tile([128, 4 * NS], BF16, tag="kt")
        for mc in range(4):
            for b in range(n):
                ps = pa.tile([128, 512], F32, tag="ps")
                for kc in range(4):
                    nc.tensor.matmul(
                        ps[:, :S],
                        wk[:, (kc * 4 + mc) * 128:(kc * 4 + mc + 1) * 128],
                        te[:, kc * NS + b * S:kc * NS + (b + 1) * S],
                        start=(kc == 0), stop=(kc == 3))
                _plain_copy(nc, mc * n + b,
                            kt[:, mc * NS + b * S:mc * NS + (b + 1) * S],
                            ps[:, :S])

        # V with ones column per head: [128(s-sub), n_sc x (8h x 65)]
        vsb = vsp.tile([128, n_sc * 520], BF16, tag="vsb")
        for sc in range(n_sc):
            ps = pa.tile([128, 512], F32, tag="ps")
            for kc in range(4):
                nc.tensor.matmul(
                    ps[:, :], te[:, kc * NS + sc * 128:kc * NS + sc * 128 + 128],
                    wv[:, kc * 512:(kc + 1) * 512],
                    start=(kc == 0), stop=(kc == 3))
            dst = vsb[:, sc * 520:(sc + 1) * 520].rearrange("p (h c) -> p h c", h=8)
            _plain_copy(nc, sc, dst[:, :, 0:64],
                        ps[:, :].rearrange("p (h c) -> p h c", h=8))
            nc.vector.memset(dst[:, :, 64:65], 1.0)

        def xd_produce_b(b):
            """Decoder gate precompute for example b (PE-heavy; overlaps the
            next example's attention, which is ACT-heavy)."""
            for tbk in range(T // TN):
                st = dsb.tile([128, 16 * TN], BF16, tag="xd_stage")
                sv = st[:, :].rearrange("p (g t) -> p g t", g=16)
                for gc in range(16):
                    ps = pa.tile([128, 512], F32, tag="ps")
                    nc.tensor.matmul(
                        ps[:, :TN], m1[:, gc * 128:(gc + 1) * 128],
                        melT[:, b * T + tbk * TN:b * T + (tbk + 1) * TN],
                        start=True, stop=False)
                    for hc in range(4):
                        nc.tensor.matmul(
                            ps[:, :TN],
                            m2[:, (hc * 16 + gc) * 128:(hc * 16 + gc + 1) * 128],
                            ctx[:, hc * NT + b * T + tbk * TN:
                                hc * NT + b * T + (tbk + 1) * TN],
                            start=False, stop=(hc == 3))
                    _plain_copy(nc, b * 16 + gc, sv[:, gc, :], ps[:, :TN])
                nc.sync.dma_start(xd_ap[:, :, b, tbk * TN:(tbk + 1) * TN],
                                  sv[:, :, :])

        # per (example, head): scoresT -> exp -> ctx + sums -> scale
        nsc_b = S // 128  # s-chunks per example
        for b in range(n):
            if do_xd and b > 0:
                xd_produce_b(b - 1)
            for h in range(NH):
                hc, hr = h // 2, (h % 2) * 64
                et = etp.tile([128, nsc_b * T], BF16, tag="et")
                for tt in range(T // TN):
                    qs = qt[hr:hr + 64,
                            hc * NT + b * T + tt * TN:hc * NT + b * T + (tt + 1) * TN]
                    for scl in range(nsc_b):
                        ps = pa.tile([128, TN], F32, tag="ps")
                        nc.tensor.matmul(
                            ps[:, :],
                            kt[hr:hr + 64,
                               hc * NS + b * S + scl * 128:hc * NS + b * S + scl * 128 + 128],
                            qs, start=True, stop=True)
                        nc.scalar.activation(
                            et[:, scl * T + tt * TN:scl * T + (tt + 1) * TN],
                            ps[:, :], AF.Exp)
                cps = [pa.tile([65, TN], F32, tag="ps", name=f"cps{tt}")
                       for tt in range(T // TN)]
                for scl in range(nsc_b):
                    lhs = vsb[:, (b * nsc_b + scl) * 520 + h * 65:
                              (b * nsc_b + scl) * 520 + (h + 1) * 65]
                    for tt in range(T // TN):
                        nc.tensor.matmul(cps[tt][:, :], lhs,
                                         et[:, scl * T + tt * TN:scl * T + (tt + 1) * TN],
                                         start=(scl == 0), stop=(scl == nsc_b - 1))
                for tt in range(T // TN):
                    rc = cxp.tile([1, TN], F32, tag="recip")
                    nc.vector.reciprocal(rc[:, :], cps[tt][64:65, :])
                    rb = pa.tile([64, TN], F32, tag="ps")
                    nc.tensor.matmul(rb[:, :], ones[:, :], rc[:, :],
                                     start=True, stop=True)
                    c0 = cxp.tile([64, TN], F32, tag="ctx_unsc")
                    _plain_copy(nc, b * NH + h + tt, c0[:, :], cps[tt][0:64, :])
                    sc1 = cxp.tile([64, TN], F32, tag="ctx_scaled")
                    nc.vector.tensor_mul(sc1[:, :], c0[:, :], rb[:, :])
                    nc.vector.tensor_scalar_add(
                        ctx[hr:hr + 64,
                            hc * NT + b * T + tt * TN:hc * NT + b * T + (tt + 1) * TN],
                        sc1[:, :], vb[:, h:h + 1])
        if do_xd:
            xd_produce_b(n - 1)


# ---------------------------------------------------------------------------
# host wrapper
# ---------------------------------------------------------------------------

def prep_host(inputs, S, T, n_per_core, ncores):
    f32 = np.float32
    emb = np.asarray(inputs["emb"], f32)
    idx = np.asarray(inputs["phoneme_indices"]).astype(np.int64)
    mel = np.asarray(inputs["mel_specs"], f32)

    bias_f = np.asarray(inputs["enc_bih_f"], f32) + np.asarray(inputs["enc_bhh_f"], f32)
    bias_b = np.asarray(inputs["enc_bih_b"], f32) + np.asarray(inputs["enc_bhh_b"], f32)
    tab_f = (emb @ np.asarray(inputs["enc_Wih_f"], f32).T + bias_f)[:, GPERM]
    tab_b = (emb @ np.asarray(inputs["enc_Wih_b"], f32).T + bias_b)[:, GPERM]

    tproj_W = np.asarray(inputs["tproj_W"], f32)
    tproj_b = np.asarray(inputs["tproj_b"], f32)
    Wq, Wk, Wv = np.split(np.asarray(inputs["attn_in_W"], f32), 3, axis=0)
    bq, bk, bv = np.split(np.asarray(inputs["attn_in_b"], f32), 3)
    mpw = np.asarray(inputs["mproj_in_W"], f32)
    mpb = np.asarray(inputs["mproj_in_b"], f32)
    aow = np.asarray(inputs["attn_out_W"], f32)
    aob = np.asarray(inputs["attn_out_b"], f32)
    dWih = np.asarray(inputs["dec_Wih"], f32)[GPERM]
    dbias = (np.asarray(inputs["dec_bih"], f32)
             + np.asarray(inputs["dec_bhh"], f32))[GPERM]
    mow = np.asarray(inputs["mproj_out_W"], f32)
    mob = np.asarray(inputs["mproj_out_b"], f32)

    scale = f32(1.0) / np.sqrt(f32(HD))
    QW = (Wq @ mpw) * scale
    qb_ = (bq + Wq @ mpb) * scale
    vb_ = bv                       # tproj_b folded into te; bk softmax-invariant
    W1, W2 = dWih[:, :H], dWih[:, H:]
    M1 = W1 @ mpw
    M2 = W2 @ aow
    dbias_ = dbias + W1 @ mpb + W2 @ aob

    whh_f_p = np.asarray(inputs["enc_Whh_f"], f32)[GPERM]
    whh_b_p = np.asarray(inputs["enc_Whh_b"], f32)[GPERM]
    whh_d_p = np.asarray(inputs["dec_Whh"], f32)[GPERM]

    def bf(a):
        return np.ascontiguousarray(a.astype(BF))

    common = {
        "ident": bf(np.eye(128, dtype=f32)),
        "tab_f": bf(_lhsT_tiles(tab_f)),
        "tab_b": bf(_lhsT_tiles(tab_b)),
        "whh_f": bf(_lhsT_tiles(whh_f_p.T)),
        "whh_b": bf(_lhsT_tiles(whh_b_p.T)),
        "whh_d": bf(_lhsT_tiles(whh_d_p.T)),
        "wt_f": bf(_lhsT_tiles(tproj_W[:, :H].T)),
        "wt_b": bf(_lhsT_tiles(tproj_W[:, H:].T)),
        "tb": np.ascontiguousarray(tproj_b.reshape(4, 128).T.astype(f32)),
        "wk": bf(_lhsT_tiles(Wk.T)),
        "wv": bf(Wv.T.reshape(4, 128, 512).transpose(1, 0, 2).reshape(128, 2048)),
        "vb": np.ascontiguousarray(vb_.reshape(8, 64).T.astype(f32)),
        "qw": bf(np.concatenate([QW.T, qb_.reshape(1, 512)], axis=0)),
        "m1": bf(np.concatenate([M1.T, dbias_.reshape(1, 2048)], axis=0)),
        "m2": bf(M2.T.reshape(4, 128, 2048).transpose(1, 0, 2).reshape(128, 4 * 2048)),
        "mo": bf(mow.T.reshape(4, 128, 80).transpose(1, 0, 2).reshape(128, 320)),
        "mob": np.ascontiguousarray(mob.reshape(80, 1).astype(f32)),
    }

    shifted = np.concatenate([np.zeros_like(mel[:, :1]), mel[:, :-1]], axis=1)

    in_maps = []
    for c in range(ncores):
        exs = list(range(c * n_per_core, (c + 1) * n_per_core))
        ohf = np.zeros((VOCAB, n_per_core * S), f32)
        ohb = np.zeros((VOCAB, n_per_core * S), f32)
        cols = np.arange(S)
        for bi, e in enumerate(exs):
            ohf[idx[e, :S], bi * S + cols] = 1.0
            ohb[idx[e, S - 1 - cols], bi * S + cols] = 1.0
        melTc = np.ones((MEL + 1, n_per_core * T), f32)
        for bi, e in enumerate(exs):
            melTc[:MEL, bi * T:(bi + 1) * T] = shifted[e, :T].T
        m = dict(common)
        m["oh_f"] = bf(np.concatenate([ohf[:128], ohf[128:]], axis=1))
        m["oh_b"] = bf(np.concatenate([ohb[:128], ohb[128:]], axis=1))
        m["melT"] = bf(melTc)
        in_maps.append(m)
    return in_maps


_PROG_CACHE = {}


def run(inputs, S, T, n, ncores, trace=False):
    key = (S, T, n)
    nc = _PROG_CACHE.get(key)
    if nc is None:
        nc = build_program(S=S, T=T, n=n)
        _PROG_CACHE[key] = nc
    in_maps = prep_host(inputs, S, T, n, ncores)
    res = bass_utils.run_bass_kernel_spmd(
        nc, in_maps, core_ids=list(range(ncores)), trace=trace)
    Bt = n * ncores
    out = np.zeros((Bt, T, MEL), np.float32)
    for c in range(ncores):
        o = np.asarray(res.results[c]["out"])
        for bi in range(n):
            out[c * n + bi] = o[:, bi * T:(bi + 1) * T].T
    return out, res


def kernel(**inputs):
    out, _ = run(inputs, S_FULL, T_FULL, B_FULL // NCORES, NCORES)
    return out


# revision 13
# speedup vs baseline: 13.8597x; 1.0680x over previous
# Trainium2 Bass kernel for nn_KokoroModel (text->mel seq2seq, LSTM enc/dec + MHA).
#
# Sharding: data-parallel over batch, 4 examples/core on 8 cores, weights
# replicated. Host-side weight-only fusions (model reparametrizations):
#   TAB  = emb @ enc_Wih.T            (embedding gather -> one-hot matmul)
#   QW   = (Wq @ mproj_in_W) / 8      (mel proj + Q proj + attn scale)
#   M1   = dec_Wih[:, :H] @ mproj_in_W
#   M2   = dec_Wih[:, H:] @ attn_out_W
#   K/V biases absorb tproj_b; decoder gate bias absorbs mproj_in/attn_out biases.
# Softmax normalization is deferred: exp-scores stay unnormalized, row-sums come
# from a ones-column appended to V, context is scaled by 1/sum afterwards.
#
# LSTM recurrence design (the serial bottleneck, 2024 steps):
#  - Gate types go to four separate PSUM banks so ACT can read a finished
#    bank while PE writes the next (PSUM collisions are bank-level). PE
#    emission order is [g, i, f, o]: the c-update chain (needs g,i,f) hides
#    under the o-bank matmuls; only sigmoid(o) + final h-mul trail the stream.
#  - xg (input gates) are precomputed to DRAM laid out [128,(type,gc,b,t)]
#    (t contiguous) and streamed in PF-step slabs, prefetched one slab ahead;
#    an identity-matmul accumulates xg into each PSUM bank (no DVE add).
#  - h_t is written by the final DVE mul directly (strided) into the SBUF
#    h-history buffer in (kc, b, t) layout; next-step matmuls read strided
#    columns from there. No per-step DMA.
#  - Encoder fwd/bwd chains interleave per tick: one direction's gate algebra
#    overlaps the other's matmul stream.

import numpy as np
import ml_dtypes

import concourse.bass as bass
import concourse.tile as tile
from concourse import bacc, mybir
from concourse import bass_utils

F32 = mybir.dt.float32
BF16 = mybir.dt.bfloat16
AF = mybir.ActivationFunctionType
BF = ml_dtypes.bfloat16

VOCAB, MEL, H = 256, 80, 512
NH, HD = 8, 64
B_FULL, S_FULL, T_FULL = 32, 512, 1000
NCORES = 8
PF = 16  # xg slab length (steps per prefetch)

# gate-type permutation: torch order [i,f,g,o] -> ours [i,f,o,g] (type-major)
GPERM = np.r_[0:H, H:2 * H, 3 * H:4 * H, 2 * H:3 * H]


def _lhsT_tiles(w, kp=128, mp=128):
    """w: [K, M] -> [kp, (K//kp)*(M//mp)*mp]; tile (kc,mc) at cols
    (kc*(M//mp)+mc)*mp."""
    K, M = w.shape
    nk, nm = K // kp, M // mp
    return np.ascontiguousarray(
        w.reshape(nk, kp, nm, mp).transpose(1, 0, 2, 3).reshape(kp, nk * nm * mp))


# ---------------------------------------------------------------------------
# device program
# ---------------------------------------------------------------------------

def build_program(S=S_FULL, T=T_FULL, n=B_FULL // NCORES, stages=5):
    nc = bacc.Bacc("TRN2", target_bir_lowering=False, debug=False)

    NS, NT = n * S, n * T
    TN = T // 2       # mel matmul N-tile
    assert TN <= 512

    d = {}

    def din(name, shape, dt):
        d[name] = nc.dram_tensor(name, list(shape), dt, kind="ExternalInput")

    din("ident", (128, 128), BF16)
    din("oh_f", (128, 2 * NS), BF16)
    din("oh_b", (128, 2 * NS), BF16)
    din("tab_f", (128, 2 * 16 * 128), BF16)   # stationary tiles (vc, gc)
    din("tab_b", (128, 2 * 16 * 128), BF16)
    din("whh_f", (128, 4 * 16 * 128), BF16)   # stationary tiles (kc, gc)
    din("whh_b", (128, 4 * 16 * 128), BF16)
    din("whh_d", (128, 4 * 16 * 128), BF16)
    din("wt_f", (128, 4 * 4 * 128), BF16)
    din("wt_b", (128, 4 * 4 * 128), BF16)
    din("tb", (128, 4), F32)
    din("wk", (128, 4 * 4 * 128), BF16)
    din("wv", (128, 4 * 512), BF16)
    din("vb", (64, 8), F32)
    din("qw", (81, 4 * 128), BF16)
    din("melT", (81, NT), BF16)
    din("m1", (81, 16 * 128), BF16)           # stationary tiles (gc)
    din("m2", (128, 4 * 16 * 128), BF16)      # stationary tiles (hc, gc)
    din("mo", (128, 4 * 80), BF16)
    din("mob", (80, 1), F32)

    out = nc.dram_tensor("out", [80, NT], F32, kind="ExternalOutput")

    # xg layout: [128, (type4*gc4, b, t)] -- t contiguous per (type,gc,b)
    xg_f = nc.dram_tensor("xg_f", [128, 16 * NS], BF16, kind="Internal")
    xg_b = nc.dram_tensor("xg_b", [128, 16 * NS], BF16, kind="Internal")
    xg_d = nc.dram_tensor("xg_d", [128, 16 * NT], BF16, kind="Internal")

    with tile.TileContext(nc) as tc:
        _body(tc, nc, d, out, xg_f, xg_b, xg_d, S, T, n, NS, NT, TN, stages)

    nc.compile()
    return nc


def _plain_copy(nc, i, dst, src):
    if i % 2 == 0:
        nc.scalar.copy(dst, src)
    else:
        nc.vector.tensor_copy(dst, src)


def _load(nc, pool, d, name, shape, dt):
    t_ = pool.tile(list(shape), dt, tag=name)
    nc.sync.dma_start(t_[:, :], d[name].ap())
    return t_


def _xg_produce(tc, nc, psp, sbp, tab, oh, xg_dram, S, n):
    """xg[p, (g, b, t)] = (TAB_tile.T @ onehot)[gate, token]; bias folded in.
    Gates land on partitions; DRAM runs are contiguous in t."""
    NS = n * S
    TB = min(S, 512)
    nsb = S // TB
    xg_ap = xg_dram.ap().rearrange("p (g b t) -> p g b t", g=16, b=n)
    i = 0
    for b in range(n):
        st = sbp.tile([128, 16 * S], BF16, tag="xg_stage")
        sv = st[:, :].rearrange("p (g t) -> p g t", g=16)
        for gc in range(16):
            for tb in range(nsb):
                ps = psp.tile([128, 512], F32, tag="ps")
                col0 = b * S + tb * TB
                for vc in range(2):
                    nc.tensor.matmul(
                        ps[:, :TB],
                        tab[:, (vc * 16 + gc) * 128:(vc * 16 + gc + 1) * 128],
                        oh[:, vc * NS + col0:vc * NS + col0 + TB],
                        start=(vc == 0), stop=(vc == 1))
                _plain_copy(nc, i, sv[:, gc, tb * TB:(tb + 1) * TB], ps[:, :TB])
                i += 1
        nc.sync.dma_start(xg_ap[:, :, b, :], sv[:, :, :])


class _LstmChain:
    """State + per-step emission for one LSTM recurrence."""

    def __init__(self, nc, pools, T_steps, n, xg_dram, whh, ob, col_of_t,
                 ident, tag, use_ident=True):
        self.nc = nc
        self.use_ident = use_ident
        (self.psp, self.stp, self.actp, self.prodp, self.tcp, self.xgp) = pools
        self.T_steps, self.n = T_steps, n
        self.xg_ap = xg_dram.ap().rearrange("p (g b t) -> p g b t", g=16, b=n)
        self.whh = whh
        self.ob = ob.rearrange("p (kc b t) -> p kc b t", kc=4, b=n)
        self.col_of_t = col_of_t
        self.ident = ident
        self.tag = tag
        self.gw = 4 * n
        self.pending = []
        st = self.stp.tile([128, 2 * self.gw], F32, tag=f"st_{tag}")
        nc.vector.memset(st[:, self.gw:2 * self.gw], 0.0)   # c = 0
        self.st = st
        self.xgs = None

    def prefetch(self, t0):
        if t0 >= self.T_steps:
            return
        pf = min(PF, self.T_steps - t0)
        xgs = self.xgp.tile([128, 16 * self.n * PF], BF16,
                            tag=f"xgs_{self.tag}")
        xv = xgs[:, :].rearrange("p (g b t) -> p g b t", g=16, t=PF)
        self.nc.sync.dma_start(xv[:, :, :, :pf], self.xg_ap[:, :, :, t0:t0 + pf])
        self.pending.append(xgs)

    def step(self, t):
        nc, n, gw = self.nc, self.n, self.gw
        if t % PF == 0:
            self.xgs = self.pending.pop(0)
            self.prefetch(t + PF)
        toff = t % PF
        xv = self.xgs[:, :].rearrange("p (g b t) -> p g b t", g=16, t=PF)

        # PE bank order [g, i, f, o]; type indices: i=0, f=1, o=2, g=3.
        # use_ident: xg accumulated into PSUM by identity-matmuls, emitted
        # first (no h dependency -> fills PE during the previous step's
        # tail). Otherwise xg is added on DVE (frees PE issue slots; only
        # used where the ALG chain is hidden, i.e. interleaved encoder).
        banks = {}
        if self.use_ident:
            for typ in (3, 0, 1, 2):
                ps = self.psp.tile([128, 512], F32, tag=f"ps{typ}_{self.tag}")
                banks[typ] = ps
                pv = ps[:, 0:gw].rearrange("p (g b) -> p g b", g=4)
                nc.tensor.matmul(
                    pv[:, :, :], self.ident[:, :],
                    xv[:, typ * 4:(typ + 1) * 4, :, toff],
                    start=True, stop=(t == 0))
        elif t > 0:
            for typ in (3, 0, 1, 2):
                ps = self.psp.tile([128, 512], F32,
                                   tag=f"ps{typ}_{self.tag}")
                banks[typ] = ps
        for typ in (3, 0, 1, 2):
            if t > 0:
                ps = banks[typ]
                hcol = self.col_of_t(t - 1)
                for gcl in range(4):
                    gg = typ * 4 + gcl
                    for kc in range(4):
                        nc.tensor.matmul(
                            ps[:, gcl * n:(gcl + 1) * n],
                            self.whh[:, (kc * 16 + gg) * 128:
                                     (kc * 16 + gg + 1) * 128],
                            self.ob[:, kc, :, hcol],
                            start=(not self.use_ident and kc == 0),
                            stop=(kc == 3))
            # gate pre-activations for this bank
            if self.use_ident:
                gin = banks[typ][:, 0:gw]
            elif t == 0:
                # no matmul contribution at t=0: gates are just xg
                gin = xv[:, typ * 4:(typ + 1) * 4, :, toff]
            else:
                gt = self.prodp.tile([128, gw], F32, tag=f"g{typ}_{self.tag}")
                nc.vector.tensor_add(
                    gt[:, :].rearrange("p (g b) -> p g b", g=4),
                    banks[typ][:, 0:gw].rearrange("p (g b) -> p g b", g=4),
                    xv[:, typ * 4:(typ + 1) * 4, :, toff])
                gin = gt[:, :]
            gv3 = gin if len(gin.shape) == 2 else None
            # ALG, interleaved so each bank is consumed while PE moves on
            if typ == 3:      # tanh(g) -> st[0:gw]
                dst = self.st[:, 0:gw]
                if gv3 is None:
                    dst = dst.rearrange("p (g b) -> p g b", g=4)
                nc.scalar.activation(dst, gin, AF.Tanh)
                self.acts = self.actp.tile([128, 3 * gw], F32,
                                           tag=f"acts_{self.tag}")
            elif typ == 0:    # sigmoid(i) -> acts[0:gw]
                dst = self.acts[:, 0:gw]
                if gv3 is None:
                    dst = dst.rearrange("p (g b) -> p g b", g=4)
                nc.scalar.activation(dst, gin, AF.Sigmoid)
            elif typ == 1:    # sigmoid(f); then c-chain (independent of o)
                dst = self.acts[:, gw:2 * gw]
                if gv3 is None:
                    dst = dst.rearrange("p (g b) -> p g b", g=4)
                nc.scalar.activation(dst, gin, AF.Sigmoid)
                prod = self.prodp.tile([128, 2 * gw], F32,
                                       tag=f"prod_{self.tag}")
                nc.vector.tensor_mul(prod[:, :], self.acts[:, 0:2 * gw],
                                     self.st[:, :])
                nc.vector.tensor_add(self.st[:, gw:2 * gw], prod[:, 0:gw],
                                     prod[:, gw:2 * gw])
                tc_ = self.tcp.tile([128, gw], F32, tag=f"tc_{self.tag}")
                nc.scalar.activation(tc_[:, :], self.st[:, gw:2 * gw], AF.Tanh)
                self.tc_ = tc_
            else:             # sigmoid(o); h = sig(o) * tanh(c) -> ob[..., t]
                dst = self.acts[:, 2 * gw:3 * gw]
                if gv3 is None:
                    dst = dst.rearrange("p (g b) -> p g b", g=4)
                nc.scalar.activation(dst, gin, AF.Sigmoid)
                nc.vector.tensor_mul(
                    self.ob[:, :, :, self.col_of_t(t)],
                    self.acts[:, 2 * gw:3 * gw].rearrange(
                        "p (kc b) -> p kc b", kc=4),
                    self.tc_[:, :].rearrange("p (kc b) -> p kc b", kc=4))


def _lstm_phase(chains, T_steps):
    for ch in chains:
        ch.prefetch(0)
    for t in range(T_steps):
        for ch in chains:
            ch.step(t)


def _body(tc, nc, d, out, xg_f, xg_b, xg_d, S, T, n, NS, NT, TN, stages=5):
    n_tn = NT // TN

    with tc.tile_pool(name="persist", bufs=1) as pp:
        te = pp.tile([128, 4 * NS], BF16, tag="te")
        ctx = pp.tile([128, 4 * NT], BF16, tag="ctx")
        ident = _load(nc, pp, d, "ident", (128, 128), BF16)

        # ---------------- phase E0: encoder gate precompute ----------------
        with tc.tile_pool(name="e0w", bufs=1) as wp, \
             tc.tile_pool(name="e0ps", bufs=4, space="PSUM") as psp, \
             tc.tile_pool(name="e0sb", bufs=2) as sbp:
            tab_f = _load(nc, wp, d, "tab_f", (128, 4096), BF16)
            tab_b = _load(nc, wp, d, "tab_b", (128, 4096), BF16)
            oh_f = _load(nc, wp, d, "oh_f", (128, 2 * NS), BF16)
            oh_b = _load(nc, wp, d, "oh_b", (128, 2 * NS), BF16)
            _xg_produce(tc, nc, psp, sbp, tab_f, oh_f, xg_f, S, n)
            _xg_produce(tc, nc, psp, sbp, tab_b, oh_b, xg_b, S, n)
        if stages <= 1:
            return

        # ---------------- phase E1: encoder recurrences (interleaved) ------
        with tc.tile_pool(name="e1w", bufs=1) as ewp, \
             tc.tile_pool(name="e1buf", bufs=1) as ebp:
            whh_f = _load(nc, ewp, d, "whh_f", (128, 8192), BF16)
            whh_b = _load(nc, ewp, d, "whh_b", (128, 8192), BF16)
            buf_f = ebp.tile([128, 4 * NS], BF16, tag="buf_f")
            buf_b = ebp.tile([128, 4 * NS], BF16, tag="buf_b")

            with tc.tile_pool(name="e1ps", bufs=1, space="PSUM") as pls, \
                 tc.tile_pool(name="e1st", bufs=1) as stp, \
                 tc.tile_pool(name="e1act", bufs=2) as actp, \
                 tc.tile_pool(name="e1prod", bufs=2) as prodp, \
                 tc.tile_pool(name="e1tc", bufs=2) as tcp, \
                 tc.tile_pool(name="e1xg", bufs=3) as xgp:
                lp = (pls, stp, actp, prodp, tcp, xgp)
                chf = _LstmChain(nc, lp, S, n, xg_f, whh_f, buf_f[:, :],
                                 lambda t: t, ident, "f", use_ident=False)
                chb = _LstmChain(nc, lp, S, n, xg_b, whh_b, buf_b[:, :],
                                 lambda t: S - 1 - t, ident, "b",
                                 use_ident=False)
                _lstm_phase([chf, chb], S)

            # ---------------- phase E2: text projection -------------------
            with tc.tile_pool(name="e2w", bufs=1) as wtp, \
                 tc.tile_pool(name="e2ps", bufs=4, space="PSUM") as ptp:
                wt_f = _load(nc, wtp, d, "wt_f", (128, 2048), BF16)
                wt_b = _load(nc, wtp, d, "wt_b", (128, 2048), BF16)
                tb = _load(nc, wtp, d, "tb", (128, 4), F32)
                for mc in range(4):
                    for b in range(n):
                        ps = ptp.tile([128, 512], F32, tag="ps")
                        for kc in range(4):
                            nc.tensor.matmul(
                                ps[:, :S],
                                wt_f[:, (kc * 4 + mc) * 128:(kc * 4 + mc + 1) * 128],
                                buf_f[:, kc * NS + b * S:kc * NS + (b + 1) * S],
                                start=(kc == 0), stop=False)
                        for kc in range(4):
                            nc.tensor.matmul(
                                ps[:, :S],
                                wt_b[:, (kc * 4 + mc) * 128:(kc * 4 + mc + 1) * 128],
                                buf_b[:, kc * NS + b * S:kc * NS + (b + 1) * S],
                                start=False, stop=(kc == 3))
                        dst = te[:, mc * NS + b * S:mc * NS + (b + 1) * S]
                        if (mc * n + b) % 2 == 0:
                            nc.scalar.activation(dst, ps[:, :S], AF.Identity,
                                                 bias=tb[:, mc:mc + 1])
                        else:
                            nc.vector.tensor_scalar_add(dst, ps[:, :S],
                                                        tb[:, mc:mc + 1])

        if stages <= 2:
            return
        # ------- phase A+X: attention, interleaved with decoder gates -----
        melT = pp.tile([81, NT], BF16, tag="melT")
        nc.sync.dma_start(melT[:, :], d["melT"].ap())
        _attention(tc, nc, d, te, ctx, melT, S, T, n, NS, NT, TN, xg_d,
                   stages)

        if stages <= 4:
            return
        # ---------------- phase D: decoder recurrence + out proj ----------
        with tc.tile_pool(name="dw", bufs=1) as dwp, \
             tc.tile_pool(name="dbuf", bufs=1) as dbp:
            whh_d = _load(nc, dwp, d, "whh_d", (128, 8192), BF16)
            dbuf = dbp.tile([128, 4 * NT], BF16, tag="dbuf")

            with tc.tile_pool(name="dps", bufs=2, space="PSUM") as pls, \
                 tc.tile_pool(name="dst", bufs=1) as stp, \
                 tc.tile_pool(name="dact", bufs=2) as actp, \
                 tc.tile_pool(name="dprod", bufs=2) as prodp, \
                 tc.tile_pool(name="dtc", bufs=2) as tcp, \
                 tc.tile_pool(name="dxg", bufs=3) as xgp:
                lp = (pls, stp, actp, prodp, tcp, xgp)
                chd = _LstmChain(nc, lp, T, n, xg_d, whh_d, dbuf[:, :],
                                 lambda t: t, ident, "d")
                _lstm_phase([chd], T)

            with tc.tile_pool(name="ow", bufs=1) as mop, \
                 tc.tile_pool(name="ops", bufs=4, space="PSUM") as pso, \
                 tc.tile_pool(name="osb", bufs=4) as sbo:
                mo = _load(nc, mop, d, "mo", (128, 320), BF16)
                mob = _load(nc, mop, d, "mob", (80, 1), F32)
                for nt in range(n_tn):
                    ps = pso.tile([80, TN], F32, tag="ps")
                    for kc in range(4):
                        nc.tensor.matmul(
                            ps[:, :], mo[:, kc * 80:(kc + 1) * 80],
                            dbuf[:, kc * NT + nt * TN:kc * NT + (nt + 1) * TN],
                            start=(kc == 0), stop=(kc == 3))
                    sb = sbo.tile([80, TN], F32, tag="out_sb")
                    nc.scalar.activation(sb[:, :], ps[:, :], AF.Identity,
                                         bias=mob[:, :])
                    nc.sync.dma_start(out.ap()[:, nt * TN:(nt + 1) * TN],
                                      sb[:, :])


def _attention(tc, nc, d, te, ctx, melT, S, T, n, NS, NT, TN, xg_d, stages):
    n_tn = NT // TN
    n_sc = NS // 128
    with tc.tile_pool(name="aw", bufs=1) as awp, \
         tc.tile_pool(name="aps", bufs=4, space="PSUM") as pa, \
         tc.tile_pool(name="aqt", bufs=1) as qtp, \
         tc.tile_pool(name="akt", bufs=1) as ktp, \
         tc.tile_pool(name="avs", bufs=1) as vsp, \
         tc.tile_pool(name="aet", bufs=2) as etp, \
         tc.tile_pool(name="actx", bufs=3) as cxp, \
         tc.tile_pool(name="xsb", bufs=1) as dsb:

        wk = _load(nc, awp, d, "wk", (128, 2048), BF16)
        wv = _load(nc, awp, d, "wv", (128, 2048), BF16)
        vb = _load(nc, awp, d, "vb", (64, 8), F32)
        qw = _load(nc, awp, d, "qw", (81, 512), BF16)
        do_xd = stages > 3
        if do_xd:
            m1 = _load(nc, awp, d, "m1", (81, 2048), BF16)
            m2 = _load(nc, awp, d, "m2", (128, 4 * 2048), BF16)
            xd_ap = xg_d.ap().rearrange("p (g b t) -> p g b t", g=16, b=n)
        ones = awp.tile([1, 64], F32, tag="ones64")
        nc.vector.memset(ones[:, :], 1.0)

        # QT sbuf-resident: [128, 4mc x NT]
        qt = qtp.tile([128, 4 * NT], BF16, tag="qt")
        for mc in range(4):
            for nt in range(n_tn):
                ps = pa.tile([128, TN], F32, tag="ps")
                nc.tensor.matmul(ps[:, :], qw[:, mc * 128:(mc + 1) * 128],
                                 melT[:, nt * TN:(nt + 1) * TN],
                                 start=True, stop=True)
                _plain_copy(nc, mc * n_tn + nt,
                            qt[:, mc * NT + nt * TN:mc * NT + (nt + 1) * TN],
                            ps[:, :])

        # KT sbuf-resident: [128, 4mc x NS]
        kt = ktp.tile([128, 4 * NS], BF16, tag="kt")
        for mc in range(4):
            for b in range(n):
                ps = pa.tile([128, 512], F32, tag="ps")
                for kc in range(4):
                    nc.tensor.matmul(
                        ps[:, :S],
                        wk[:, (kc * 4 + mc) * 128:(kc * 4 + mc + 1) * 128],
                        te[:, kc * NS + b * S:kc * NS + (b + 1) * S],
                        start=(kc == 0), stop=(kc == 3))
                _plain_copy(nc, mc * n + b,
                            kt[:, mc * NS + b * S:mc * NS + (b + 1) * S],
                            ps[:, :S])

        # V with ones column per head: [128(s-sub), n_sc x (8h x 65)]
        vsb = vsp.tile([128, n_sc * 520], BF16, tag="vsb")
        for sc in range(n_sc):
            ps = pa.tile([128, 512], F32, tag="ps")
            for kc in range(4):
                nc.tensor.matmul(
                    ps[:, :], te[:, kc * NS + sc * 128:kc * NS + sc * 128 + 128],
                    wv[:, kc * 512:(kc + 1) * 512],
                    start=(kc == 0), stop=(kc == 3))
            dst = vsb[:, sc * 520:(sc + 1) * 520].rearrange("p (h c) -> p h c", h=8)
            _plain_copy(nc, sc, dst[:, :, 0:64],
                        ps[:, :].rearrange("p (h c) -> p h c", h=8))
            nc.vector.memset(dst[:, :, 64:65], 1.0)

        def xd_produce_b(b):
            """Decoder gate precompute for example b (PE-heavy; overlaps the
            next example's attention, which is ACT-heavy)."""
            for tbk in range(T // TN):
                st = dsb.tile([128, 16 * TN], BF16, tag="xd_stage")
                sv = st[:, :].rearrange("p (g t) -> p g t", g=16)
                for gc in range(16):
                    ps = pa.tile([128, 512], F32, tag="ps")
                    nc.tensor.matmul(
                        ps[:, :TN], m1[:, gc * 128:(gc + 1) * 128],
                        melT[:, b * T + tbk * TN:b * T + (tbk + 1) * TN],
                        start=True, stop=False)
                    for hc in range(4):
                        nc.tensor.matmul(
                            ps[:, :TN],
                            m2[:, (hc * 16 + gc) * 128:(hc * 16 + gc + 1) * 128],
                            ctx[:, hc * NT + b * T + tbk * TN:
                                hc * NT + b * T + (tbk + 1) * TN],
                            start=False, stop=(hc == 3))
                    _plain_copy(nc, b * 16 + gc, sv[:, gc, :], ps[:, :TN])
                nc.sync.dma_start(xd_ap[:, :, b, tbk * TN:(tbk + 1) * TN],
                                  sv[:, :, :])

        # per (example, head): scoresT -> exp -> ctx + sums -> scale
        nsc_b = S // 128  # s-chunks per example
        for b in range(n):
            if do_xd and b > 0:
                xd_produce_b(b - 1)
            for h in range(NH):
                hc, hr = h // 2, (h % 2) * 64
                et = etp.tile([128, nsc_b * T], BF16, tag="et")
                for tt in range(T // TN):
                    qs = qt[hr:hr + 64,
                            hc * NT + b * T + tt * TN:hc * NT + b * T + (tt + 1) * TN]
                    for scl in range(nsc_b):
                        ps = pa.tile([128, TN], F32, tag="ps")
                        nc.tensor.matmul(
                            ps[:, :],
                            kt[hr:hr + 64,
                               hc * NS + b * S + scl * 128:hc * NS + b * S + scl * 128 + 128],
                            qs, start=True, stop=True)
                        nc.scalar.activation(
                            et[:, scl * T + tt * TN:scl * T + (tt + 1) * TN],
                            ps[:, :], AF.Exp)
                cps = [pa.tile([65, TN], F32, tag="ps", name=f"cps{tt}")
                       for tt in range(T // TN)]
                for scl in range(nsc_b):
                    lhs = vsb[:, (b * nsc_b + scl) * 520 + h * 65:
                              (b * nsc_b + scl) * 520 + (h + 1) * 65]
                    for tt in range(T // TN):
                        nc.tensor.matmul(cps[tt][:, :], lhs,
                                         et[:, scl * T + tt * TN:scl * T + (tt + 1) * TN],
                                         start=(scl == 0), stop=(scl == nsc_b - 1))
                for tt in range(T // TN):
                    rc = cxp.tile([1, TN], F32, tag="recip")
                    nc.vector.reciprocal(rc[:, :], cps[tt][64:65, :])
                    rb = pa.tile([64, TN], F32, tag="ps")
                    nc.tensor.matmul(rb[:, :], ones[:, :], rc[:, :],
                                     start=True, stop=True)
                    c0 = cxp.tile([64, TN], F32, tag="ctx_unsc")
                    _plain_copy(nc, b * NH + h + tt, c0[:, :], cps[tt][0:64, :])
                    sc1 = cxp.tile([64, TN], F32, tag="ctx_scaled")
                    nc.vector.tensor_mul(sc1[:, :], c0[:, :], rb[:, :])
                    nc.vector.tensor_scalar_add(
                        ctx[hr:hr + 64,
                            hc * NT + b * T + tt * TN:hc * NT + b * T + (tt + 1) * TN],
                        sc1[:, :], vb[:, h:h + 1])
        if do_xd:
            xd_produce_b(n - 1)


# ---------------------------------------------------------------------------
# host wrapper
# ---------------------------------------------------------------------------

def prep_host(inputs, S, T, n_per_core, ncores):
    f32 = np.float32
    emb = np.asarray(inputs["emb"], f32)
    idx = np.asarray(inputs["phoneme_indices"]).astype(np.int64)
    mel = np.asarray(inputs["mel_specs"], f32)

    bias_f = np.asarray(inputs["enc_bih_f"], f32) + np.asarray(inputs["enc_bhh_f"], f32)
    bias_b = np.asarray(inputs["enc_bih_b"], f32) + np.asarray(inputs["enc_bhh_b"], f32)
    tab_f = (emb @ np.asarray(inputs["enc_Wih_f"], f32).T + bias_f)[:, GPERM]
    tab_b = (emb @ np.asarray(inputs["enc_Wih_b"], f32).T + bias_b)[:, GPERM]

    tproj_W = np.asarray(inputs["tproj_W"], f32)
    tproj_b = np.asarray(inputs["tproj_b"], f32)
    Wq, Wk, Wv = np.split(np.asarray(inputs["attn_in_W"], f32), 3, axis=0)
    bq, bk, bv = np.split(np.asarray(inputs["attn_in_b"], f32), 3)
    mpw = np.asarray(inputs["mproj_in_W"], f32)
    mpb = np.asarray(inputs["mproj_in_b"], f32)
    aow = np.asarray(inputs["attn_out_W"], f32)
    aob = np.asarray(inputs["attn_out_b"], f32)
    dWih = np.asarray(inputs["dec_Wih"], f32)[GPERM]
    dbias = (np.asarray(inputs["dec_bih"], f32)
             + np.asarray(inputs["dec_bhh"], f32))[GPERM]
    mow = np.asarray(inputs["mproj_out_W"], f32)
    mob = np.asarray(inputs["mproj_out_b"], f32)

    scale = f32(1.0) / np.sqrt(f32(HD))
    QW = (Wq @ mpw) * scale
    qb_ = (bq + Wq @ mpb) * scale
    vb_ = bv                       # tproj_b folded into te; bk softmax-invariant
    W1, W2 = dWih[:, :H], dWih[:, H:]
    M1 = W1 @ mpw
    M2 = W2 @ aow
    dbias_ = dbias + W1 @ mpb + W2 @ aob

    whh_f_p = np.asarray(inputs["enc_Whh_f"], f32)[GPERM]
    whh_b_p = np.asarray(inputs["enc_Whh_b"], f32)[GPERM]
    whh_d_p = np.asarray(inputs["dec_Whh"], f32)[GPERM]

    def bf(a):
        return np.ascontiguousarray(a.astype(BF))

    common = {
        "ident": bf(np.eye(128, dtype=f32)),
        "tab_f": bf(_lhsT_tiles(tab_f)),
        "tab_b": bf(_lhsT_tiles(tab_b)),
        "whh_f": bf(_lhsT_tiles(whh_f_p.T)),
        "whh_b": bf(_lhsT_tiles(whh_b_p.T)),
        "whh_d": bf(_lhsT_tiles(whh_d_p.T)),
        "wt_f": bf(_lhsT_tiles(tproj_W[:, :H].T)),
        "wt_b": bf(_lhsT_tiles(tproj_W[:, H:].T)),
        "tb": np.ascontiguousarray(tproj_b.reshape(4, 128).T.astype(f32)),
        "wk": bf(_lhsT_tiles(Wk.T)),
        "wv": bf(Wv.T.reshape(4, 128, 512).transpose(1, 0, 2).reshape(128, 2048)),
        "vb": np.ascontiguousarray(vb_.reshape(8, 64).T.astype(f32)),
        "qw": bf(np.concatenate([QW.T, qb_.reshape(1, 512)], axis=0)),
        "m1": bf(np.concatenate([M1.T, dbias_.reshape(1, 2048)], axis=0)),
        "m2": bf(M2.T.reshape(4, 128, 2048).transpose(1, 0, 2).reshape(128, 4 * 2048)),
        "mo": bf(mow.T.reshape(4, 128, 80).transpose(1, 0, 2).reshape(128, 320)),
        "mob": np.ascontiguousarray(mob.reshape(80, 1).astype(f32)),
    }

    shifted = np.concatenate([np.zeros_like(mel[:, :1]), mel[:, :-1]], axis=1)

    in_maps = []
    for c in range(ncores):
        exs = list(range(c * n_per_core, (c + 1) * n_per_core))
        ohf = np.zeros((VOCAB, n_per_core * S), f32)
        ohb = np.zeros((VOCAB, n_per_core * S), f32)
        cols = np.arange(S)
        for bi, e in enumerate(exs):
            ohf[idx[e, :S], bi * S + cols] = 1.0
            ohb[idx[e, S - 1 - cols], bi * S + cols] = 1.0
        melTc = np.ones((MEL + 1, n_per_core * T), f32)
        for bi, e in enumerate(exs):
            melTc[:MEL, bi * T:(bi + 1) * T] = shifted[e, :T].T
        m = dict(common)
        m["oh_f"] = bf(np.concatenate([ohf[:128], ohf[128:]], axis=1))
        m["oh_b"] = bf(np.concatenate([ohb[:128], ohb[128:]], axis=1))
        m["melT"] = bf(melTc)
        in_maps.append(m)
    return in_maps


_PROG_CACHE = {}


def run(inputs, S, T, n, ncores, trace=False):
    key = (S, T, n)
    nc = _PROG_CACHE.get(key)
    if nc is None:
        nc = build_program(S=S, T=T, n=n)
        _PROG_CACHE[key] = nc
    in_maps = prep_host(inputs, S, T, n, ncores)
    res = bass_utils.run_bass_kernel_spmd(
        nc, in_maps, core_ids=list(range(ncores)), trace=trace)
    Bt = n * ncores
    out = np.zeros((Bt, T, MEL), np.float32)
    for c in range(ncores):
        o = np.asarray(res.results[c]["out"])
        for bi in range(n):
            out[c * n + bi] = o[:, bi * T:(bi + 1) * T].T
    return out, res


def kernel(**inputs):
    out, _ = run(inputs, S_FULL, T_FULL, B_FULL // NCORES, NCORES)
    return out


# revision 14
# speedup vs baseline: 14.6180x; 1.0547x over previous
# Trainium2 Bass kernel for nn_KokoroModel (text->mel seq2seq, LSTM enc/dec + MHA).
#
# Sharding: data-parallel over batch, 4 examples/core on 8 cores, weights
# replicated. Host-side weight-only fusions (model reparametrizations):
#   TAB  = emb @ enc_Wih.T            (embedding gather -> one-hot matmul)
#   QW   = (Wq @ mproj_in_W) / 8      (mel proj + Q proj + attn scale)
#   M1   = dec_Wih[:, :H] @ mproj_in_W
#   M2   = dec_Wih[:, H:] @ attn_out_W
#   K/V biases absorb tproj_b; decoder gate bias absorbs mproj_in/attn_out biases.
# Softmax normalization is deferred: exp-scores stay unnormalized, row-sums come
# from a ones-column appended to V, context is scaled by 1/sum afterwards.
#
# LSTM recurrence design (the serial bottleneck, 2024 steps):
#  - Gate types go to four separate PSUM banks so ACT can read a finished
#    bank while PE writes the next (PSUM collisions are bank-level). PE
#    emission order is [g, i, f, o]: the c-update chain (needs g,i,f) hides
#    under the o-bank matmuls; only sigmoid(o) + final h-mul trail the stream.
#  - xg (input gates) are precomputed to DRAM laid out [128,(type,gc,b,t)]
#    (t contiguous) and streamed in PF-step slabs, prefetched one slab ahead;
#    an identity-matmul accumulates xg into each PSUM bank (no DVE add).
#  - h_t is written by the final DVE mul directly (strided) into the SBUF
#    h-history buffer in (kc, b, t) layout; next-step matmuls read strided
#    columns from there. No per-step DMA.
#  - Encoder fwd/bwd chains interleave per tick: one direction's gate algebra
#    overlaps the other's matmul stream.

import numpy as np
import ml_dtypes

import concourse.bass as bass
import concourse.tile as tile
from concourse import bacc, mybir
from concourse import bass_utils

F32 = mybir.dt.float32
BF16 = mybir.dt.bfloat16
AF = mybir.ActivationFunctionType
BF = ml_dtypes.bfloat16

VOCAB, MEL, H = 256, 80, 512
NH, HD = 8, 64
B_FULL, S_FULL, T_FULL = 32, 512, 1000
NCORES = 8
PF = 16  # xg slab length (steps per prefetch)

# gate-type permutation: torch order [i,f,g,o] -> ours [i,f,o,g] (type-major)
GPERM = np.r_[0:H, H:2 * H, 3 * H:4 * H, 2 * H:3 * H]


def _lhsT_tiles(w, kp=128, mp=128):
    """w: [K, M] -> [kp, (K//kp)*(M//mp)*mp]; tile (kc,mc) at cols
    (kc*(M//mp)+mc)*mp."""
    K, M = w.shape
    nk, nm = K // kp, M // mp
    return np.ascontiguousarray(
        w.reshape(nk, kp, nm, mp).transpose(1, 0, 2, 3).reshape(kp, nk * nm * mp))


# ---------------------------------------------------------------------------
# device program
# ---------------------------------------------------------------------------

def build_program(S=S_FULL, T=T_FULL, n=B_FULL // NCORES, stages=5):
    nc = bacc.Bacc("TRN2", target_bir_lowering=False, debug=False)

    NS, NT = n * S, n * T
    TN = T // 2       # mel matmul N-tile
    assert TN <= 512

    d = {}

    def din(name, shape, dt):
        d[name] = nc.dram_tensor(name, list(shape), dt, kind="ExternalInput")

    din("ident", (128, 128), BF16)
    din("oh_f", (128, 2 * NS), BF16)
    din("oh_b", (128, 2 * NS), BF16)
    din("tab_f", (128, 2 * 16 * 128), BF16)   # stationary tiles (vc, gc)
    din("tab_b", (128, 2 * 16 * 128), BF16)
    din("whh_f", (128, 4 * 16 * 128), BF16)   # stationary tiles (kc, gc)
    din("whh_b", (128, 4 * 16 * 128), BF16)
    din("whh_d", (128, 4 * 16 * 128), BF16)
    din("wt_f", (128, 4 * 4 * 128), BF16)
    din("wt_b", (128, 4 * 4 * 128), BF16)
    din("tb", (128, 4), F32)
    din("wk", (128, 4 * 4 * 128), BF16)
    din("wv", (128, 4 * 512), BF16)
    din("vb", (64, 8), F32)
    din("qw", (81, 4 * 128), BF16)
    din("melT", (81, NT), BF16)
    din("m1", (81, 16 * 128), BF16)           # stationary tiles (gc)
    din("m2", (128, 4 * 16 * 128), BF16)      # stationary tiles (hc, gc)
    din("mo", (128, 4 * 80), BF16)
    din("mob", (80, 1), F32)

    out = nc.dram_tensor("out", [80, NT], F32, kind="ExternalOutput")

    # xg layout: [128, (type4*gc4, b, t)] -- t contiguous per (type,gc,b)
    xg_f = nc.dram_tensor("xg_f", [128, 16 * NS], BF16, kind="Internal")
    xg_b = nc.dram_tensor("xg_b", [128, 16 * NS], BF16, kind="Internal")
    xg_d = nc.dram_tensor("xg_d", [128, 16 * NT], BF16, kind="Internal")

    with tile.TileContext(nc) as tc:
        _body(tc, nc, d, out, xg_f, xg_b, xg_d, S, T, n, NS, NT, TN, stages)

    nc.compile()
    return nc


def _plain_copy(nc, i, dst, src):
    if i % 2 == 0:
        nc.scalar.copy(dst, src)
    else:
        nc.vector.tensor_copy(dst, src)


def _load(nc, pool, d, name, shape, dt):
    t_ = pool.tile(list(shape), dt, tag=name)
    nc.sync.dma_start(t_[:, :], d[name].ap())
    return t_


def _xg_produce(tc, nc, psp, sbp, tab, oh, xg_dram, S, n):
    """xg[p, (g, b, t)] = (TAB_tile.T @ onehot)[gate, token]; bias folded in.
    Gates land on partitions; DRAM runs are contiguous in t."""
    NS = n * S
    TB = min(S, 512)
    nsb = S // TB
    xg_ap = xg_dram.ap().rearrange("p (g b t) -> p g b t", g=16, b=n)
    i = 0
    for b in range(n):
        st = sbp.tile([128, 16 * S], BF16, tag="xg_stage")
        sv = st[:, :].rearrange("p (g t) -> p g t", g=16)
        for gc in range(16):
            for tb in range(nsb):
                ps = psp.tile([128, 512], F32, tag="ps")
                col0 = b * S + tb * TB
                for vc in range(2):
                    nc.tensor.matmul(
                        ps[:, :TB],
                        tab[:, (vc * 16 + gc) * 128:(vc * 16 + gc + 1) * 128],
                        oh[:, vc * NS + col0:vc * NS + col0 + TB],
                        start=(vc == 0), stop=(vc == 1))
                _plain_copy(nc, i, sv[:, gc, tb * TB:(tb + 1) * TB], ps[:, :TB])
                i += 1
        nc.sync.dma_start(xg_ap[:, :, b, :], sv[:, :, :])


class _LstmChain:
    """State + per-step emission for one LSTM recurrence."""

    def __init__(self, nc, pools, T_steps, n, xg_dram, whh, ob, col_of_t,
                 ident, tag):
        self.nc = nc
        (self.psp, self.stp, self.actp, self.prodp, self.tcp, self.xgp) = pools
        self.T_steps, self.n = T_steps, n
        self.xg_ap = xg_dram.ap().rearrange("p (g b t) -> p g b t", g=16, b=n)
        self.whh = whh
        self.ob = ob.rearrange("p (kc b t) -> p kc b t", kc=4, b=n)
        self.col_of_t = col_of_t
        self.ident = ident
        self.tag = tag
        self.gw = 4 * n
        self.pending = []
        st = self.stp.tile([128, 2 * self.gw], F32, tag=f"st_{tag}")
        nc.vector.memset(st[:, self.gw:2 * self.gw], 0.0)   # c = 0
        self.st = st
        self.xgs = None

    def prefetch(self, t0):
        if t0 >= self.T_steps:
            return
        pf = min(PF, self.T_steps - t0)
        xgs = self.xgp.tile([128, 16 * self.n * PF], BF16,
                            tag=f"xgs_{self.tag}")
        xv = xgs[:, :].rearrange("p (g b t) -> p g b t", g=16, t=PF)
        self.nc.sync.dma_start(xv[:, :, :, :pf], self.xg_ap[:, :, :, t0:t0 + pf])
        self.pending.append(xgs)

    def step(self, t):
        nc, n, gw = self.nc, self.n, self.gw
        if t % PF == 0:
            self.xgs = self.pending.pop(0)
            self.prefetch(t + PF)
        toff = t % PF
        xv = self.xgs[:, :].rearrange("p (g b t) -> p g b t", g=16, t=PF)

        # PE bank order [g, i, f, o]; type indices: i=0, f=1, o=2, g=3.
        # All four xg ident-matmuls first: they don't depend on h, so they
        # fill the PE while the previous step's tail (sig_o/mul_h) finishes.
        banks = {}
        for typ in (3, 0, 1, 2):
            ps = self.psp.tile([128, 512], F32, tag=f"ps{typ}_{self.tag}")
            banks[typ] = ps
            pv = ps[:, 0:gw].rearrange("p (g b) -> p g b", g=4)
            nc.tensor.matmul(
                pv[:, :, :], self.ident[:, :],
                xv[:, typ * 4:(typ + 1) * 4, :, toff],
                start=True, stop=(t == 0))
        for typ in (3, 0, 1, 2):
            ps = banks[typ]
            if t > 0:
                hcol = self.col_of_t(t - 1)
                for gcl in range(4):
                    gg = typ * 4 + gcl
                    for kc in range(4):
                        nc.tensor.matmul(
                            ps[:, gcl * n:(gcl + 1) * n],
                            self.whh[:, (kc * 16 + gg) * 128:
                                     (kc * 16 + gg + 1) * 128],
                            self.ob[:, kc, :, hcol],
                            start=False, stop=(kc == 3))
            # ALG, interleaved so each bank is consumed while PE moves on
            if typ == 3:      # tanh(g) -> st[0:gw]
                nc.scalar.activation(self.st[:, 0:gw], ps[:, 0:gw], AF.Tanh)
                self.acts = self.actp.tile([128, 3 * gw], F32,
                                           tag=f"acts_{self.tag}")
            elif typ == 0:    # sigmoid(i) -> acts[0:gw]
                nc.scalar.activation(self.acts[:, 0:gw], ps[:, 0:gw],
                                     AF.Sigmoid)
            elif typ == 1:    # sigmoid(f); then c-chain (independent of o)
                nc.scalar.activation(self.acts[:, gw:2 * gw], ps[:, 0:gw],
                                     AF.Sigmoid)
                prod = self.prodp.tile([128, 2 * gw], F32,
                                       tag=f"prod_{self.tag}")
                nc.vector.tensor_mul(prod[:, :], self.acts[:, 0:2 * gw],
                                     self.st[:, :])
                nc.vector.tensor_add(self.st[:, gw:2 * gw], prod[:, 0:gw],
                                     prod[:, gw:2 * gw])
                tc_ = self.tcp.tile([128, gw], F32, tag=f"tc_{self.tag}")
                nc.scalar.activation(tc_[:, :], self.st[:, gw:2 * gw], AF.Tanh)
                self.tc_ = tc_
            else:             # sigmoid(o); h = sig(o) * tanh(c) -> ob[..., t]
                nc.scalar.activation(self.acts[:, 2 * gw:3 * gw], ps[:, 0:gw],
                                     AF.Sigmoid)
                nc.vector.tensor_mul(
                    self.ob[:, :, :, self.col_of_t(t)],
                    self.acts[:, 2 * gw:3 * gw].rearrange(
                        "p (kc b) -> p kc b", kc=4),
                    self.tc_[:, :].rearrange("p (kc b) -> p kc b", kc=4))


def _lstm_phase(chains, T_steps):
    for ch in chains:
        ch.prefetch(0)
    for t in range(T_steps):
        for ch in chains:
            ch.step(t)


def _body(tc, nc, d, out, xg_f, xg_b, xg_d, S, T, n, NS, NT, TN, stages=5):
    n_tn = NT // TN

    with tc.tile_pool(name="persist", bufs=1) as pp:
        te = pp.tile([128, 4 * NS], BF16, tag="te")
        ctx = pp.tile([128, 4 * NT], BF16, tag="ctx")
        ident = _load(nc, pp, d, "ident", (128, 128), BF16)

        # ---------------- phase E0: encoder gate precompute ----------------
        with tc.tile_pool(name="e0w", bufs=1) as wp, \
             tc.tile_pool(name="e0ps", bufs=4, space="PSUM") as psp, \
             tc.tile_pool(name="e0sb", bufs=2) as sbp:
            tab_f = _load(nc, wp, d, "tab_f", (128, 4096), BF16)
            tab_b = _load(nc, wp, d, "tab_b", (128, 4096), BF16)
            oh_f = _load(nc, wp, d, "oh_f", (128, 2 * NS), BF16)
            oh_b = _load(nc, wp, d, "oh_b", (128, 2 * NS), BF16)
            _xg_produce(tc, nc, psp, sbp, tab_f, oh_f, xg_f, S, n)
            _xg_produce(tc, nc, psp, sbp, tab_b, oh_b, xg_b, S, n)
        if stages <= 1:
            return

        # ---------------- phase E1: encoder recurrences (interleaved) ------
        with tc.tile_pool(name="e1w", bufs=1) as ewp, \
             tc.tile_pool(name="e1buf", bufs=1) as ebp:
            whh_f = _load(nc, ewp, d, "whh_f", (128, 8192), BF16)
            whh_b = _load(nc, ewp, d, "whh_b", (128, 8192), BF16)
            buf_f = ebp.tile([128, 4 * NS], BF16, tag="buf_f")
            buf_b = ebp.tile([128, 4 * NS], BF16, tag="buf_b")

            with tc.tile_pool(name="e1ps", bufs=1, space="PSUM") as pls, \
                 tc.tile_pool(name="e1st", bufs=1) as stp, \
                 tc.tile_pool(name="e1act", bufs=2) as actp, \
                 tc.tile_pool(name="e1prod", bufs=2) as prodp, \
                 tc.tile_pool(name="e1tc", bufs=2) as tcp, \
                 tc.tile_pool(name="e1xg", bufs=3) as xgp:
                lp = (pls, stp, actp, prodp, tcp, xgp)
                chf = _LstmChain(nc, lp, S, n, xg_f, whh_f, buf_f[:, :],
                                 lambda t: t, ident, "f")
                chb = _LstmChain(nc, lp, S, n, xg_b, whh_b, buf_b[:, :],
                                 lambda t: S - 1 - t, ident, "b")
                _lstm_phase([chf, chb], S)

            # ---------------- phase E2: text projection -------------------
            with tc.tile_pool(name="e2w", bufs=1) as wtp, \
                 tc.tile_pool(name="e2ps", bufs=4, space="PSUM") as ptp:
                wt_f = _load(nc, wtp, d, "wt_f", (128, 2048), BF16)
                wt_b = _load(nc, wtp, d, "wt_b", (128, 2048), BF16)
                tb = _load(nc, wtp, d, "tb", (128, 4), F32)
                for mc in range(4):
                    for b in range(n):
                        ps = ptp.tile([128, 512], F32, tag="ps")
                        for kc in range(4):
                            nc.tensor.matmul(
                                ps[:, :S],
                                wt_f[:, (kc * 4 + mc) * 128:(kc * 4 + mc + 1) * 128],
                                buf_f[:, kc * NS + b * S:kc * NS + (b + 1) * S],
                                start=(kc == 0), stop=False)
                        for kc in range(4):
                            nc.tensor.matmul(
                                ps[:, :S],
                                wt_b[:, (kc * 4 + mc) * 128:(kc * 4 + mc + 1) * 128],
                                buf_b[:, kc * NS + b * S:kc * NS + (b + 1) * S],
                                start=False, stop=(kc == 3))
                        dst = te[:, mc * NS + b * S:mc * NS + (b + 1) * S]
                        if (mc * n + b) % 2 == 0:
                            nc.scalar.activation(dst, ps[:, :S], AF.Identity,
                                                 bias=tb[:, mc:mc + 1])
                        else:
                            nc.vector.tensor_scalar_add(dst, ps[:, :S],
                                                        tb[:, mc:mc + 1])

        if stages <= 2:
            return
        # ------- phase A+X: attention, interleaved with decoder gates -----
        melT = pp.tile([81, NT], BF16, tag="melT")
        nc.sync.dma_start(melT[:, :], d["melT"].ap())
        _attention(tc, nc, d, te, ctx, melT, S, T, n, NS, NT, TN, xg_d,
                   stages)

        if stages <= 4:
            return
        # ---------------- phase D: decoder recurrence + out proj ----------
        with tc.tile_pool(name="dw", bufs=1) as dwp, \
             tc.tile_pool(name="dbuf", bufs=1) as dbp:
            whh_d = _load(nc, dwp, d, "whh_d", (128, 8192), BF16)
            dbuf = dbp.tile([128, 4 * NT], BF16, tag="dbuf")

            with tc.tile_pool(name="dps", bufs=2, space="PSUM") as pls, \
                 tc.tile_pool(name="dst", bufs=1) as stp, \
                 tc.tile_pool(name="dact", bufs=2) as actp, \
                 tc.tile_pool(name="dprod", bufs=2) as prodp, \
                 tc.tile_pool(name="dtc", bufs=2) as tcp, \
                 tc.tile_pool(name="dxg", bufs=3) as xgp:
                lp = (pls, stp, actp, prodp, tcp, xgp)
                chd = _LstmChain(nc, lp, T, n, xg_d, whh_d, dbuf[:, :],
                                 lambda t: t, ident, "d")
                _lstm_phase([chd], T)

            with tc.tile_pool(name="ow", bufs=1) as mop, \
                 tc.tile_pool(name="ops", bufs=4, space="PSUM") as pso, \
                 tc.tile_pool(name="osb", bufs=4) as sbo:
                mo = _load(nc, mop, d, "mo", (128, 320), BF16)
                mob = _load(nc, mop, d, "mob", (80, 1), F32)
                for nt in range(n_tn):
                    ps = pso.tile([80, TN], F32, tag="ps")
                    for kc in range(4):
                        nc.tensor.matmul(
                            ps[:, :], mo[:, kc * 80:(kc + 1) * 80],
                            dbuf[:, kc * NT + nt * TN:kc * NT + (nt + 1) * TN],
                            start=(kc == 0), stop=(kc == 3))
                    sb = sbo.tile([80, TN], F32, tag="out_sb")
                    nc.scalar.activation(sb[:, :], ps[:, :], AF.Identity,
                                         bias=mob[:, :])
                    nc.sync.dma_start(out.ap()[:, nt * TN:(nt + 1) * TN],
                                      sb[:, :])


def _attention(tc, nc, d, te, ctx, melT, S, T, n, NS, NT, TN, xg_d, stages):
    n_tn = NT // TN
    n_sc = NS // 128
    with tc.tile_pool(name="aw", bufs=1) as awp, \
         tc.tile_pool(name="aps", bufs=4, space="PSUM") as pa, \
         tc.tile_pool(name="aqt", bufs=1) as qtp, \
         tc.tile_pool(name="akt", bufs=1) as ktp, \
         tc.tile_pool(name="avs", bufs=1) as vsp, \
         tc.tile_pool(name="aet", bufs=2) as etp, \
         tc.tile_pool(name="actx", bufs=3) as cxp, \
         tc.tile_pool(name="xsb", bufs=1) as dsb:

        wk = _load(nc, awp, d, "wk", (128, 2048), BF16)
        wv = _load(nc, awp, d, "wv", (128, 2048), BF16)
        vb = _load(nc, awp, d, "vb", (64, 8), F32)
        qw = _load(nc, awp, d, "qw", (81, 512), BF16)
        do_xd = stages > 3
        if do_xd:
            m1 = _load(nc, awp, d, "m1", (81, 2048), BF16)
            m2 = _load(nc, awp, d, "m2", (128, 4 * 2048), BF16)
            xd_ap = xg_d.ap().rearrange("p (g b t) -> p g b t", g=16, b=n)
        ones = awp.tile([1, 64], F32, tag="ones64")
        nc.vector.memset(ones[:, :], 1.0)

        # QT sbuf-resident: [128, 4mc x NT]
        qt = qtp.tile([128, 4 * NT], BF16, tag="qt")
        for mc in range(4):
            for nt in range(n_tn):
                ps = pa.tile([128, TN], F32, tag="ps")
                nc.tensor.matmul(ps[:, :], qw[:, mc * 128:(mc + 1) * 128],
                                 melT[:, nt * TN:(nt + 1) * TN],
                                 start=True, stop=True)
                _plain_copy(nc, mc * n_tn + nt,
                            qt[:, mc * NT + nt * TN:mc * NT + (nt + 1) * TN],
                            ps[:, :])

        # KT sbuf-resident: [128, 4mc x NS]
        kt = ktp.tile([128, 4 * NS], BF16, tag="kt")
        for mc in range(4):
            for b in range(n):
                ps = pa.tile([128, 512], F32, tag="ps")
                for kc in range(4):
                    nc.tensor.matmul(
                        ps[:, :S],
                        wk[:, (kc * 4 + mc) * 128:(kc * 4 + mc + 1) * 128],
                        te[:, kc * NS + b * S:kc * NS + (b + 1) * S],
                        start=(kc == 0), stop=(kc == 3))
                _plain_copy(nc, mc * n + b,
                            kt[:, mc * NS + b * S:mc * NS + (b + 1) * S],
                            ps[:, :S])

        # V with ones column per head: [128(s-sub), n_sc x (8h x 65)]
        vsb = vsp.tile([128, n_sc * 520], BF16, tag="vsb")
        for sc in range(n_sc):
            ps = pa.tile([128, 512], F32, tag="ps")
            for kc in range(4):
                nc.tensor.matmul(
                    ps[:, :], te[:, kc * NS + sc * 128:kc * NS + sc * 128 + 128],
                    wv[:, kc * 512:(kc + 1) * 512],
                    start=(kc == 0), stop=(kc == 3))
            dst = vsb[:, sc * 520:(sc + 1) * 520].rearrange("p (h c) -> p h c", h=8)
            _plain_copy(nc, sc, dst[:, :, 0:64],
                        ps[:, :].rearrange("p (h c) -> p h c", h=8))
            nc.vector.memset(dst[:, :, 64:65], 1.0)

        def xd_produce_b(b):
            """Decoder gate precompute for example b (PE-heavy; overlaps the
            next example's attention, which is ACT-heavy)."""
            for tbk in range(T // TN):
                st = dsb.tile([128, 16 * TN], BF16, tag="xd_stage")
                sv = st[:, :].rearrange("p (g t) -> p g t", g=16)
                for gc in range(16):
                    ps = pa.tile([128, 512], F32, tag="ps")
                    nc.tensor.matmul(
                        ps[:, :TN], m1[:, gc * 128:(gc + 1) * 128],
                        melT[:, b * T + tbk * TN:b * T + (tbk + 1) * TN],
                        start=True, stop=False)
                    for hc in range(4):
                        nc.tensor.matmul(
                            ps[:, :TN],
                            m2[:, (hc * 16 + gc) * 128:(hc * 16 + gc + 1) * 128],
                            ctx[:, hc * NT + b * T + tbk * TN:
                                hc * NT + b * T + (tbk + 1) * TN],
                            start=False, stop=(hc == 3))
                    _plain_copy(nc, b * 16 + gc, sv[:, gc, :], ps[:, :TN])
                nc.sync.dma_start(xd_ap[:, :, b, tbk * TN:(tbk + 1) * TN],
                                  sv[:, :, :])

        # per (example, head): scoresT -> exp -> ctx + sums -> scale
        nsc_b = S // 128  # s-chunks per example
        for b in range(n):
            if do_xd and b > 0:
                xd_produce_b(b - 1)
            for h in range(NH):
                hc, hr = h // 2, (h % 2) * 64
                et = etp.tile([128, nsc_b * T], BF16, tag="et")
                for tt in range(T // TN):
                    qs = qt[hr:hr + 64,
                            hc * NT + b * T + tt * TN:hc * NT + b * T + (tt + 1) * TN]
                    for scl in range(nsc_b):
                        ps = pa.tile([128, TN], F32, tag="ps")
                        nc.tensor.matmul(
                            ps[:, :],
                            kt[hr:hr + 64,
                               hc * NS + b * S + scl * 128:hc * NS + b * S + scl * 128 + 128],
                            qs, start=True, stop=True)
                        nc.scalar.activation(
                            et[:, scl * T + tt * TN:scl * T + (tt + 1) * TN],
                            ps[:, :], AF.Exp)
                cps = [pa.tile([65, TN], F32, tag="ps", name=f"cps{tt}")
                       for tt in range(T // TN)]
                for scl in range(nsc_b):
                    lhs = vsb[:, (b * nsc_b + scl) * 520 + h * 65:
                              (b * nsc_b + scl) * 520 + (h + 1) * 65]
                    for tt in range(T // TN):
                        nc.tensor.matmul(cps[tt][:, :], lhs,
                                         et[:, scl * T + tt * TN:scl * T + (tt + 1) * TN],
                                         start=(scl == 0), stop=(scl == nsc_b - 1))
                for tt in range(T // TN):
                    rc = cxp.tile([1, TN], F32, tag="recip")
                    nc.vector.reciprocal(rc[:, :], cps[tt][64:65, :])
                    rb = pa.tile([64, TN], F32, tag="ps")
                    nc.tensor.matmul(rb[:, :], ones[:, :], rc[:, :],
                                     start=True, stop=True)
                    c0 = cxp.tile([64, TN], F32, tag="ctx_unsc")
                    _plain_copy(nc, b * NH + h + tt, c0[:, :], cps[tt][0:64, :])
                    sc1 = cxp.tile([64, TN], F32, tag="ctx_scaled")
                    nc.vector.tensor_mul(sc1[:, :], c0[:, :], rb[:, :])
                    nc.vector.tensor_scalar_add(
                        ctx[hr:hr + 64,
                            hc * NT + b * T + tt * TN:hc * NT + b * T + (tt + 1) * TN],
                        sc1[:, :], vb[:, h:h + 1])
        if do_xd:
            xd_produce_b(n - 1)


# ---------------------------------------------------------------------------
# host wrapper
# ---------------------------------------------------------------------------

def prep_host(inputs, S, T, n_per_core, ncores):
    f32 = np.float32
    emb = np.asarray(inputs["emb"], f32)
    idx = np.asarray(inputs["phoneme_indices"]).astype(np.int64)
    mel = np.asarray(inputs["mel_specs"], f32)

    bias_f = np.asarray(inputs["enc_bih_f"], f32) + np.asarray(inputs["enc_bhh_f"], f32)
    bias_b = np.asarray(inputs["enc_bih_b"], f32) + np.asarray(inputs["enc_bhh_b"], f32)
    tab_f = (emb @ np.asarray(inputs["enc_Wih_f"], f32).T + bias_f)[:, GPERM]
    tab_b = (emb @ np.asarray(inputs["enc_Wih_b"], f32).T + bias_b)[:, GPERM]

    tproj_W = np.asarray(inputs["tproj_W"], f32)
    tproj_b = np.asarray(inputs["tproj_b"], f32)
    Wq, Wk, Wv = np.split(np.asarray(inputs["attn_in_W"], f32), 3, axis=0)
    bq, bk, bv = np.split(np.asarray(inputs["attn_in_b"], f32), 3)
    mpw = np.asarray(inputs["mproj_in_W"], f32)
    mpb = np.asarray(inputs["mproj_in_b"], f32)
    aow = np.asarray(inputs["attn_out_W"], f32)
    aob = np.asarray(inputs["attn_out_b"], f32)
    dWih = np.asarray(inputs["dec_Wih"], f32)[GPERM]
    dbias = (np.asarray(inputs["dec_bih"], f32)
             + np.asarray(inputs["dec_bhh"], f32))[GPERM]
    mow = np.asarray(inputs["mproj_out_W"], f32)
    mob = np.asarray(inputs["mproj_out_b"], f32)

    scale = f32(1.0) / np.sqrt(f32(HD))
    QW = (Wq @ mpw) * scale
    qb_ = (bq + Wq @ mpb) * scale
    vb_ = bv                       # tproj_b folded into te; bk softmax-invariant
    W1, W2 = dWih[:, :H], dWih[:, H:]
    M1 = W1 @ mpw
    M2 = W2 @ aow
    dbias_ = dbias + W1 @ mpb + W2 @ aob

    whh_f_p = np.asarray(inputs["enc_Whh_f"], f32)[GPERM]
    whh_b_p = np.asarray(inputs["enc_Whh_b"], f32)[GPERM]
    whh_d_p = np.asarray(inputs["dec_Whh"], f32)[GPERM]

    def bf(a):
        return np.ascontiguousarray(a.astype(BF))

    common = {
        "ident": bf(np.eye(128, dtype=f32)),
        "tab_f": bf(_lhsT_tiles(tab_f)),
        "tab_b": bf(_lhsT_tiles(tab_b)),
        "whh_f": bf(_lhsT_tiles(whh_f_p.T)),
        "whh_b": bf(_lhsT_tiles(whh_b_p.T)),
        "whh_d": bf(_lhsT_tiles(whh_d_p.T)),
        "wt_f": bf(_lhsT_tiles(tproj_W[:, :H].T)),
        "wt_b": bf(_lhsT_tiles(tproj_W[:, H:].T)),
        "tb": np.ascontiguousarray(tproj_b.reshape(4, 128).T.astype(f32)),
        "wk": bf(_lhsT_tiles(Wk.T)),
        "wv": bf(Wv.T.reshape(4, 128, 512).transpose(1, 0, 2).reshape(128, 2048)),
        "vb": np.ascontiguousarray(vb_.reshape(8, 64).T.astype(f32)),
        "qw": bf(np.concatenate([QW.T, qb_.reshape(1, 512)], axis=0)),
        "m1": bf(np.concatenate([M1.T, dbias_.reshape(1, 2048)], axis=0)),
        "m2": bf(M2.T.reshape(4, 128, 2048).transpose(1, 0, 2).reshape(128, 4 * 2048)),
        "mo": bf(mow.T.reshape(4, 128, 80).transpose(1, 0, 2).reshape(128, 320)),
        "mob": np.ascontiguousarray(mob.reshape(80, 1).astype(f32)),
    }

    shifted = np.concatenate([np.zeros_like(mel[:, :1]), mel[:, :-1]], axis=1)

    in_maps = []
    for c in range(ncores):
        exs = list(range(c * n_per_core, (c + 1) * n_per_core))
        ohf = np.zeros((VOCAB, n_per_core * S), f32)
        ohb = np.zeros((VOCAB, n_per_core * S), f32)
        cols = np.arange(S)
        for bi, e in enumerate(exs):
            ohf[idx[e, :S], bi * S + cols] = 1.0
            ohb[idx[e, S - 1 - cols], bi * S + cols] = 1.0
        melTc = np.ones((MEL + 1, n_per_core * T), f32)
        for bi, e in enumerate(exs):
            melTc[:MEL, bi * T:(bi + 1) * T] = shifted[e, :T].T
        m = dict(common)
        m["oh_f"] = bf(np.concatenate([ohf[:128], ohf[128:]], axis=1))
        m["oh_b"] = bf(np.concatenate([ohb[:128], ohb[128:]], axis=1))
        m["melT"] = bf(melTc)
        in_maps.append(m)
    return in_maps


_PROG_CACHE = {}


def run(inputs, S, T, n, ncores, trace=False):
    key = (S, T, n)
    nc = _PROG_CACHE.get(key)
    if nc is None:
        nc = build_program(S=S, T=T, n=n)
        _PROG_CACHE[key] = nc
    in_maps = prep_host(inputs, S, T, n, ncores)
    res = bass_utils.run_bass_kernel_spmd(
        nc, in_maps, core_ids=list(range(ncores)), trace=trace)
    Bt = n * ncores
    out = np.zeros((Bt, T, MEL), np.float32)
    for c in range(ncores):
        o = np.asarray(res.results[c]["out"])
        for bi in range(n):
            out[c * n + bi] = o[:, bi * T:(bi + 1) * T].T
    return out, res


def kernel(**inputs):
    out, _ = run(inputs, S_FULL, T_FULL, B_FULL // NCORES, NCORES)
    return out
